# revision 1
# baseline (speedup 1.0000x reference)
"""GCN (3-layer graph conv + 3-layer MLP head) on 8 TRN2 NeuronCores.

Strategy (graph/1D-row parallel, per sharding hint):
  - Nodes are row-sharded across the 8 cores (6250 rows each).
  - Per layer: local GEMM support = g_prev @ W (node shard), AllGather the
    [50000,128] fp16 support table to every core, then each core aggregates
    its destination rows: for each 128-row destination block, gather the
    neighbor rows (dma_gather, int16 indices, table split at row 32768 so
    indices fit int16), build a one-hot scatter matrix S[e,dst]=val[e] on
    DVE from a host-precomputed (dst,val) stream, and accumulate
    aggT[feat,dst] += msgs[e,feat].T @ S[e,dst] on the tensor engine with
    f32 PSUM. Bias+ReLU+fp16-cast happens on ACT straight out of PSUM.
  - Everything stays feature-major (gT = [feat, node]) so no transposes are
    ever needed; the FC head runs the same way and the [2, n] logits are
    transposed back on the host.

Numerics: fp16 storage / f32 accumulation -> ~1.2e-3 norm rel err vs the
f32 reference (validated offline in numpy).
"""

import numpy as np

import concourse.bass as bass
import concourse.bacc as bacc
import concourse.mybir as mybir
import concourse.tile as tile
from concourse.bass_utils import run_bass_kernel_spmd

FP16 = mybir.dt.float16
F32 = mybir.dt.float32
I16 = mybir.dt.int16

N_NODES = 50000
N_CORES = 8
D = 128
SPLIT = 32768  # int16 gather-index limit: table rows >= SPLIT use a 2nd base


# ---------------------------------------------------------------------------
# Host-side schedule construction
# ---------------------------------------------------------------------------
class _Sched:
    pass


def _prepare(row, col, vals, n_nodes, ncores, split, gsz):
    """Sort/pad edges into an SPMD-uniform static schedule.

    Returns (sched, per_core) where per_core[c] holds idx/dst/val arrays.
    """
    shard = n_nodes // ncores
    nb = (shard + 127) // 128

    core = row // shard
    lb = (row % shard) // 128
    dst = (row % shard) % 128
    half = (col >= split).astype(np.int64)

    order = np.lexsort((col, half, lb, core))
    core_s, lb_s, dst_s, half_s = core[order], lb[order], dst[order], half[order]
    col_s, val_s = col[order], vals[order]

    # composite key for boundary lookup
    key = (core_s * nb + lb_s) * 2 + half_s
    bounds = np.searchsorted(key, np.arange(ncores * nb * 2 + 1))

    def cnt(c, b, h):
        k = (c * nb + b) * 2 + h
        return bounds[k + 1] - bounds[k]

    # chunks per (block, half): shared across cores (SPMD)
    CH = np.zeros((nb, 2), np.int64)
    for b in range(nb):
        for h in range(2):
            m = max(cnt(c, b, h) for c in range(ncores))
            CH[b, h] = (m + 127) // 128
        if CH[b, 0] + CH[b, 1] == 0:
            CH[b, 0] = 1  # keep >=1 chunk so PSUM gets initialized

    # gather groups of gsz blocks; chunk order: (group, half, block, chunk)
    groups = []
    tot_ch = 0
    for g0 in range(0, nb, gsz):
        blocks = list(range(g0, min(g0 + gsz, nb)))
        gi = _Sched()
        gi.blocks = []
        gi.C = [0, 0]
        gi.ch0 = [0, 0]
        binfo = {b: {} for b in blocks}
        for h in range(2):
            gi.ch0[h] = tot_ch
            loff = 0
            for b in blocks:
                binfo[b][h] = (loff, int(CH[b, h]), tot_ch)
                loff += int(CH[b, h])
                tot_ch += int(CH[b, h])
            gi.C[h] = loff
        for b in blocks:
            gi.blocks.append((b, binfo[b][0], binfo[b][1]))
        groups.append(gi)

    sched = _Sched()
    sched.shard, sched.nb, sched.tot_ch, sched.groups = shard, nb, tot_ch, groups
    sched.split = split

    # per-core padded idx/dst/val arrays in the same chunk order
    per_core = []
    for c in range(ncores):
        idx = np.zeros(tot_ch * 128, np.int16)
        dstv = np.zeros(tot_ch * 128, np.float32)
        valv = np.zeros(tot_ch * 128, np.float32)
        for gi in groups:
            for b, lohh, hih in gi.blocks:
                for h, (loff, chn, ch0) in ((0, lohh), (1, hih)):
                    if chn == 0:
                        continue
                    k = (c * nb + b) * 2 + h
                    s, e = bounds[k], bounds[k + 1]
                    n = e - s
                    o = ch0 * 128
                    if n > 0:
                        cc = col_s[s:e] - (split if h else 0)
                        idx[o : o + n] = cc.astype(np.int16)
                        dstv[o : o + n] = dst_s[s:e].astype(np.float32)
                        valv[o : o + n] = val_s[s:e].astype(np.float32)
        pc = _Sched()
        pc.idx_sb = np.tile(np.ascontiguousarray(idx.reshape(-1, 16).T), (8, 1))
        pc.dst_sb = np.ascontiguousarray(dstv.reshape(tot_ch, 128).T)
        pc.val_sb = np.ascontiguousarray(valv.reshape(tot_ch, 128).T)
        per_core.append(pc)
    return sched, per_core


# ---------------------------------------------------------------------------
# Device program
# ---------------------------------------------------------------------------
def _build(sched, n_nodes, ncores, enable_asserts=False):
    nb, shard, tot_ch, split = sched.nb, sched.shard, sched.tot_ch, sched.split
    npad = nb * 128
    nhi = n_nodes - split

    nc = bacc.Bacc(
        "TRN2",
        target_bir_lowering=False,
        debug=False,
        enable_asserts=enable_asserts,
        num_devices=ncores,
    )

    xT_d = nc.declare_dram_parameter("xT", [128, npad], FP16, isOutput=False)
    idx_d = nc.declare_dram_parameter("idx", [128, tot_ch * 8], I16, isOutput=False)
    dst_d = nc.declare_dram_parameter("dst", [128, tot_ch], F32, isOutput=False)
    val_d = nc.declare_dram_parameter("val", [128, tot_ch], F32, isOutput=False)
    iota_d = nc.declare_dram_parameter("iota", [128, 128], FP16, isOutput=False)
    w_d = nc.declare_dram_parameter("w", [128, 3, 128], FP16, isOutput=False)
    b_d = nc.declare_dram_parameter("b", [128, 3], F32, isOutput=False)
    fw1_d = nc.declare_dram_parameter("fw1", [128, 3, 128], FP16, isOutput=False)
    fb1_d = nc.declare_dram_parameter("fb1", [128, 1], F32, isOutput=False)
    fw2_d = nc.declare_dram_parameter("fw2", [128, 64], FP16, isOutput=False)
    fb2_d = nc.declare_dram_parameter("fb2", [64, 1], F32, isOutput=False)
    fw3_d = nc.declare_dram_parameter("fw3", [64, 2], FP16, isOutput=False)
    fb3_d = nc.declare_dram_parameter("fb3", [2, 1], F32, isOutput=False)
    out_d = nc.declare_dram_parameter("out", [2, npad], F32, isOutput=True)

    Relu = mybir.ActivationFunctionType.Relu
    Copy = mybir.ActivationFunctionType.Copy
    Ident = mybir.ActivationFunctionType.Identity
    iseq = mybir.AluOpType.is_equal
    mult = mybir.AluOpType.mult

    with tile.TileContext(nc) as tc:
        with (
            tc.tile_pool(name="const", bufs=1) as cpool,
            tc.tile_pool(name="dram", bufs=1, space="DRAM") as dpool,
            tc.tile_pool(name="work", bufs=3) as wpool,
            tc.tile_pool(name="sbuild", bufs=6) as spool,
            tc.tile_pool(name="psum", bufs=2, space="PSUM") as ppool,
        ):
            sup_ts = [
                dpool.tile([shard, 128], FP16, name=f"sup_sh{l}", tag=f"sup_sh{l}")
                for l in range(3)
            ]
            tbl_ts = [
                dpool.tile([n_nodes, 128], FP16, addr_space="Shared", name=f"tbl{l}", tag=f"tbl{l}")
                for l in range(3)
            ]

            def load(d, shape, dtype, name):
                t = cpool.tile(list(shape), dtype, name=name)
                nc.sync.dma_start(t[:], d[:])
                return t

            xT = load(xT_d, [128, npad], FP16, "xT")
            idxs = load(idx_d, [128, tot_ch * 8], I16, "idxs")
            dstv = load(dst_d, [128, tot_ch], F32, "dstv")
            valv = load(val_d, [128, tot_ch], F32, "valv")
            iota = load(iota_d, [128, 128], FP16, "iota")
            w = load(w_d, [128, 3, 128], FP16, "w")
            bl = load(b_d, [128, 3], F32, "bl")
            fw1 = load(fw1_d, [128, 3, 128], FP16, "fw1")
            fb1 = load(fb1_d, [128, 1], F32, "fb1")
            fw2 = load(fw2_d, [128, 64], FP16, "fw2")
            fb2 = load(fb2_d, [64, 1], F32, "fb2")
            fw3 = load(fw3_d, [64, 2], FP16, "fw3")
            fb3 = load(fb3_d, [2, 1], F32, "fb3")

            gT = [cpool.tile([128, npad], FP16, name=f"gT{l}") for l in range(3)]
            outT = cpool.tile([2, npad], F32, name="outT")

            prev = xT
            for l in range(3):
                sup_t = sup_ts[l]
                tbl_t = tbl_ts[l]
                # ---- local GEMM: support = g_prev @ W_l (node-major psum) --
                for ib in range(nb):
                    ps = ppool.tile([128, 128], F32, tag="sup", name="ps_sup")
                    nc.tensor.matmul(
                        ps[:],
                        prev[:, ib * 128 : (ib + 1) * 128],
                        w[:, l, :],
                        start=True,
                        stop=True,
                    )
                    sup_sb = wpool.tile([128, 128], FP16, tag="sup_sb", name="sup_sb")
                    nc.scalar.activation(sup_sb[:], ps[:], Copy)
                    rows = min(128, shard - ib * 128)
                    nc.sync.dma_start(
                        sup_t[ib * 128 : ib * 128 + rows, :], sup_sb[:rows, :]
                    )

                # ---- AllGather the support table ---------------------------
                nc.gpsimd.collective_compute(
                    "AllGather",
                    mybir.AluOpType.bypass,
                    replica_groups=[list(range(ncores))],
                    ins=[sup_t.opt()],
                    outs=[tbl_t.opt()],
                )

                # ---- gather + segment-sum per destination block ------------
                for gi in sched.groups:
                    mt = {}
                    for h in range(2):
                        C = gi.C[h]
                        if C == 0:
                            continue
                        m = wpool.tile(
                            [128, C * 128], FP16, tag=f"msgs{h}", name=f"msgs{h}", bufs=2
                        )
                        base, span = (0, min(split, n_nodes)) if h == 0 else (split, nhi)
                        m3d = m[:].rearrange("p (c e) -> p c e", e=128)
                        # cap per-call num_idxs (large single calls hang on HW)
                        MAXG = 6
                        for c0 in range(0, C, MAXG):
                            cn = min(MAXG, C - c0)
                            nc.gpsimd.dma_gather(
                                out_ap=m3d[:, c0 : c0 + cn, :],
                                in_ap=tbl_t[base : base + span, :],
                                idxs_ap=idxs[
                                    :,
                                    (gi.ch0[h] + c0) * 8 : (gi.ch0[h] + c0 + cn) * 8,
                                ],
                                num_idxs=cn * 128,
                                num_idxs_reg=cn * 128,
                                elem_size=128,
                            )
                        mt[h] = m
                    for b, lohh, hih in gi.blocks:
                        ps = ppool.tile([128, 128], F32, tag="agg", name="ps_agg")
                        total = lohh[1] + hih[1]
                        k = 0
                        for h, (loff, chn, ch0) in ((0, lohh), (1, hih)):
                            if chn == 0:
                                continue
                            m3 = mt[h][:].rearrange("p (c e) -> p c e", e=128)
                            for i in range(chn):
                                S = spool.tile([128, 128], FP16, tag="S", name="S")
                                nc.vector.tensor_scalar(
                                    S[:],
                                    iota[:],
                                    dstv[:, ch0 + i : ch0 + i + 1],
                                    valv[:, ch0 + i : ch0 + i + 1],
                                    iseq,
                                    mult,
                                )
                                nc.tensor.matmul(
                                    ps[:],
                                    m3[:, loff + i, :],
                                    S[:],
                                    start=(k == 0),
                                    stop=(k == total - 1),
                                )
                                k += 1
                        nc.scalar.activation(
                            gT[l][:, b * 128 : (b + 1) * 128],
                            ps[:],
                            Relu,
                            bias=bl[:, l : l + 1],
                        )
                prev = gT[l]

            # ---- FC head (all feature-major) -------------------------------
            for ib in range(nb):
                sl = slice(ib * 128, (ib + 1) * 128)
                ps1 = ppool.tile([128, 128], F32, tag="fc1", name="ps_fc1", bufs=1)
                for j in range(3):
                    nc.tensor.matmul(
                        ps1[:], fw1[:, j, :], gT[j][:, sl], start=(j == 0), stop=(j == 2)
                    )
                h1 = wpool.tile([128, 128], FP16, tag="h1", name="h1")
                nc.scalar.activation(h1[:], ps1[:], Relu, bias=fb1[:, 0:1])
                ps2 = ppool.tile([64, 128], F32, tag="fc2", name="ps_fc2", bufs=1)
                nc.tensor.matmul(ps2[:], fw2[:], h1[:], start=True, stop=True)
                h2 = wpool.tile([64, 128], FP16, tag="h2", name="h2")
                nc.scalar.activation(h2[:], ps2[:], Relu, bias=fb2[:])
                ps3 = ppool.tile([2, 128], F32, tag="fc3", name="ps_fc3", bufs=1)
                nc.tensor.matmul(ps3[:], fw3[:], h2[:], start=True, stop=True)
                nc.scalar.activation(outT[:, sl], ps3[:], Ident, bias=fb3[:])

            nc.sync.dma_start(out_d[:], outT[:])

    nc.compile()
    return nc


# ---------------------------------------------------------------------------
# Input packing
# ---------------------------------------------------------------------------
def _in_maps(inputs, sched, per_core, n_nodes, ncores):
    shard, npad = sched.shard, sched.nb * 128
    X = np.asarray(inputs["input_feature"], np.float32)
    xTs = []
    for c in range(ncores):
        xt = np.zeros((128, npad), np.float16)
        xt[:, :shard] = X[c * shard : (c + 1) * shard].T.astype(np.float16)
        xTs.append(xt)

    f16 = lambda a: np.ascontiguousarray(np.asarray(a, np.float32).astype(np.float16))
    f32 = lambda a: np.ascontiguousarray(np.asarray(a, np.float32))
    com = {
        "iota": np.ascontiguousarray(
            np.broadcast_to(np.arange(128, dtype=np.float16), (128, 128))
        ),
        "w": np.stack([f16(inputs[k]) for k in ("W1", "W2", "W3")], axis=1),
        "b": np.stack([f32(inputs[k]) for k in ("b1", "b2", "b3")], axis=1),
        "fw1": np.ascontiguousarray(
            f16(inputs["fcW1"]).reshape(3, 128, 128).transpose(1, 0, 2)
        ),
        "fb1": f32(inputs["fcb1"]).reshape(128, 1),
        "fw2": f16(inputs["fcW2"]),
        "fb2": f32(inputs["fcb2"]).reshape(64, 1),
        "fw3": f16(inputs["fcW3"]),
        "fb3": f32(inputs["fcb3"]).reshape(2, 1),
    }
    maps = []
    for c in range(ncores):
        m = dict(com)
        m["xT"] = xTs[c]
        m["idx"] = per_core[c].idx_sb
        m["dst"] = per_core[c].dst_sb
        m["val"] = per_core[c].val_sb
        maps.append(m)
    return maps


def _postprocess(results, sched, ncores):
    shard = sched.shard
    outs = [np.asarray(results[c]["out"], np.float32)[:, :shard].T for c in range(ncores)]
    return np.ascontiguousarray(np.concatenate(outs, axis=0))


# ---------------------------------------------------------------------------
# Public entry point
# ---------------------------------------------------------------------------
_CACHE = {}


def _run(inputs, n_nodes, ncores, split, gsz, runner=None, enable_asserts=False, trace=False):
    row = np.asarray(inputs["adj_row"]).astype(np.int64)
    col = np.asarray(inputs["adj_col"]).astype(np.int64)
    vals = np.asarray(inputs["adj_vals"], np.float32)
    sched, per_core = _prepare(row, col, vals, n_nodes, ncores, split, gsz)
    nc = _build(sched, n_nodes, ncores, enable_asserts=enable_asserts)
    maps = _in_maps(inputs, sched, per_core, n_nodes, ncores)
    _CACHE["nc"], _CACHE["maps"] = nc, maps
    if runner is None:
        res = run_bass_kernel_spmd(nc, maps, list(range(ncores)), trace=trace)
        results = res.results
        _CACHE["last_bench"] = res
    else:
        results = runner(nc, maps)
    return _postprocess(results, sched, ncores)


def kernel(**inputs):
    return _run(inputs, N_NODES, N_CORES, SPLIT, gsz=7)



# revision 5
# speedup vs baseline: 1242.2876x; 1242.2876x over previous
"""GCN on 8 TRN2 cores — v2: quadrant-chunked AllGather, pipelined layers.

vs v1: the support table is AllGathered in Q=4 row-quadrants (each its own
Shared tile < 32768 rows, so int16 gather indices need no base-split), and
the next layer's local GEMM + AllGather chunk are emitted as soon as the
destination blocks they need are aggregated — the collective transfers hide
under the current layer's gather/aggregation instead of serializing.
"""

import numpy as np

import concourse.bass as bass
import concourse.bacc as bacc
import concourse.mybir as mybir
import concourse.tile as tile
from concourse.bass_utils import run_bass_kernel_spmd

FP16 = mybir.dt.float16
F32 = mybir.dt.float32
I16 = mybir.dt.int16

N_NODES = 50000
N_CORES = 8
D = 128
NQ = 2        # AllGather row-chunks per layer (2: lowest chunk padding)
GSZ = 7       # dst blocks per gather group
MAXG = 6      # chunks per dma_gather call (ring capacity / packet limits)
SCRATCH = 32768
NQUEUES = 4
SPOOL_BUFS = 16


class _S:
    pass


def _prepare(row, col, vals, n_nodes, ncores, Q=NQ, gsz=GSZ):
    shard = n_nodes // ncores
    nb = (shard + 127) // 128
    ng = (nb + gsz - 1) // gsz

    qblocks = np.array_split(np.arange(nb), Q)
    q_b0 = [int(qb[0]) for qb in qblocks]
    q_b1 = [int(qb[-1]) + 1 for qb in qblocks]          # exclusive block end
    q_r0 = [b0 * 128 for b0 in q_b0]
    q_r1 = [min(b1 * 128, shard) for b1 in q_b1]
    rq = [r1 - r0 for r0, r1 in zip(q_r0, q_r1)]        # real rows per quadrant

    c = row // shard
    r = row % shard
    lb = r // 128
    dst = r % 128
    lbg = lb // gsz

    c_s = col // shard
    r_s = col % shard
    q_s = np.searchsorted(np.asarray(q_r0[1:]), r_s, side="right")
    loc = c_s * np.asarray(rq)[q_s] + (r_s - np.asarray(q_r0)[q_s])
    assert loc.max() < 32768

    order = np.lexsort((col, lb, q_s, lbg, c))
    c_o, lb_o, dst_o, q_o = c[order], lb[order], dst[order], q_s[order]
    loc_o, val_o = loc[order], vals[order]

    kk = ((c_o * ng + lb_o // gsz) * Q + q_o) * nb + lb_o
    cnt = np.zeros((ncores, nb, Q), np.int64)
    np.add.at(cnt, (c_o, lb_o, q_o), 1)
    CH = np.maximum.reduce(
        [(cnt[cc] + 127) // 128 for cc in range(ncores)])   # [nb, Q]
    for b in range(nb):
        if CH[b].sum() == 0:
            CH[b, 0] = 1

    groups = []
    tot_ch = 0
    for g0 in range(0, nb, gsz):
        blocks = list(range(g0, min(g0 + gsz, nb)))
        gi = _S()
        gi.runs = []      # per q: (ch0_abs, C, [(b, loff, chn), ...])
        for q in range(Q):
            loff = 0
            binfo = []
            ch0 = tot_ch
            for b in blocks:
                chn = int(CH[b, q])
                binfo.append((b, loff, chn))
                loff += chn
            tot_ch += loff
            gi.runs.append((ch0, loff, binfo))
        gi.blocks = blocks
        groups.append(gi)

    sched = _S()
    sched.shard, sched.nb, sched.ng, sched.Q, sched.gsz = shard, nb, ng, Q, gsz
    sched.tot_ch, sched.groups = tot_ch, groups
    sched.q_b0, sched.q_b1, sched.q_r0, sched.q_r1, sched.rq = q_b0, q_b1, q_r0, q_r1, rq
    # group index after which each quadrant's dst blocks are fully aggregated
    sched.q_done_g = [min(ng - 1, (b1 + gsz - 1) // gsz - 1) for b1 in q_b1]

    # chunk-order bounds per (core, group, q, block)
    kk_sorted_idx = np.arange(len(kk))  # kk already sorted by construction
    per_core = []
    for cc in range(ncores):
        idx = np.zeros(tot_ch * 128, np.int16)
        dstv = np.zeros(tot_ch * 128, np.float32)
        valv = np.zeros(tot_ch * 128, np.float32)
        for gi in groups:
            for q in range(Q):
                ch0, C, binfo = gi.runs[q]
                for b, loff, chn in binfo:
                    if chn == 0:
                        continue
                    k2 = ((cc * ng + b // gsz) * Q + q) * nb + b
                    s, e = np.searchsorted(kk, [k2, k2 + 1])
                    n = e - s
                    o = (ch0 + loff) * 128
                    if n > 0:
                        idx[o:o+n] = loc_o[s:e].astype(np.int16)
                        dstv[o:o+n] = dst_o[s:e].astype(np.float32)
                        valv[o:o+n] = val_o[s:e].astype(np.float32)
        pc = _S()
        pc.idx_sb = np.tile(np.ascontiguousarray(idx.reshape(-1, 16).T), (8, 1))
        pc.dst_sb = np.ascontiguousarray(dstv.reshape(tot_ch, 128).T)
        pc.val_sb = np.ascontiguousarray(valv.reshape(tot_ch, 128).T)
        per_core.append(pc)
    return sched, per_core


def _build(sched, n_nodes, ncores, enable_asserts=False, passes=1):
    nb, shard, tot_ch = sched.nb, sched.shard, sched.tot_ch
    Q, rq = sched.Q, sched.rq
    npad = nb * 128

    nc = bacc.Bacc(
        "TRN2",
        target_bir_lowering=False,
        debug=False,
        enable_asserts=enable_asserts,
        num_devices=ncores,
        dynamic_dma_scratch_size=SCRATCH,
        num_swdge_queues=NQUEUES,
    )

    xT_d = nc.declare_dram_parameter("xT", [128, npad], FP16, isOutput=False)
    idx_d = nc.declare_dram_parameter("idx", [128, tot_ch * 8], I16, isOutput=False)
    dst_d = nc.declare_dram_parameter("dst", [128, tot_ch], F32, isOutput=False)
    val_d = nc.declare_dram_parameter("val", [128, tot_ch], F32, isOutput=False)
    iota_d = nc.declare_dram_parameter("iota", [128, 128], FP16, isOutput=False)
    w_d = nc.declare_dram_parameter("w", [128, 3, 128], FP16, isOutput=False)
    b_d = nc.declare_dram_parameter("b", [128, 3], F32, isOutput=False)
    fw1_d = nc.declare_dram_parameter("fw1", [128, 3, 128], FP16, isOutput=False)
    fb1_d = nc.declare_dram_parameter("fb1", [128, 1], F32, isOutput=False)
    fw2_d = nc.declare_dram_parameter("fw2", [128, 64], FP16, isOutput=False)
    fb2_d = nc.declare_dram_parameter("fb2", [64, 1], F32, isOutput=False)
    fw3_d = nc.declare_dram_parameter("fw3", [64, 2], FP16, isOutput=False)
    fb3_d = nc.declare_dram_parameter("fb3", [2, 1], F32, isOutput=False)
    out_d = nc.declare_dram_parameter("out", [2, npad], F32, isOutput=True)

    Relu = mybir.ActivationFunctionType.Relu
    Copy = mybir.ActivationFunctionType.Copy
    Ident = mybir.ActivationFunctionType.Identity
    iseq = mybir.AluOpType.is_equal
    mult = mybir.AluOpType.mult

    qstate = [0]

    with tile.TileContext(nc) as tc:
        with (
            tc.tile_pool(name="const", bufs=1) as cpool,
            tc.tile_pool(name="dram", bufs=1, space="DRAM") as dpool,
            tc.tile_pool(name="work", bufs=3) as wpool,
            tc.tile_pool(name="sbuild", bufs=SPOOL_BUFS) as spool,
            tc.tile_pool(name="psum", bufs=2, space="PSUM") as ppool,
        ):
            def load(d, shape, dtype, name):
                t = cpool.tile(list(shape), dtype, name=name)
                nc.sync.dma_start(t[:], d[:])
                return t

            xT = load(xT_d, [128, npad], FP16, "xT")
            idxs = load(idx_d, [128, tot_ch * 8], I16, "idxs")
            dstv = load(dst_d, [128, tot_ch], F32, "dstv")
            valv = load(val_d, [128, tot_ch], F32, "valv")
            iota = load(iota_d, [128, 128], FP16, "iota")
            w = load(w_d, [128, 3, 128], FP16, "w")
            bl = load(b_d, [128, 3], F32, "bl")
            fw1 = load(fw1_d, [128, 3, 128], FP16, "fw1")
            fb1 = load(fb1_d, [128, 1], F32, "fb1")
            fw2 = load(fw2_d, [128, 64], FP16, "fw2")
            fb2 = load(fb2_d, [64, 1], F32, "fb2")
            fw3 = load(fw3_d, [64, 2], FP16, "fw3")
            fb3 = load(fb3_d, [2, 1], F32, "fb3")

            for _pass in range(passes):
                gT = [cpool.tile([128, npad], FP16, name=f"gT{l}", tag=f"gT{l}")
                      for l in range(3)]
                outT = cpool.tile([2, npad], F32, tag="outT", name="outT")
                supq = [[dpool.tile([rq[q], 128], FP16,
                                    name=f"sup{l}q{q}_{_pass}",
                                    tag=f"sup{l}q{q}_{_pass}")
                         for q in range(Q)] for l in range(3)]
                tblq = [[dpool.tile([ncores * rq[q], 128], FP16,
                                    addr_space="Shared",
                                    name=f"tbl{l}q{q}_{_pass}",
                                    tag=f"tbl{l}q{q}_{_pass}")
                         for q in range(Q)] for l in range(3)]

                def emit_A(l, q):
                    """sup GEMMs for quadrant q of layer l, then AllGather it."""
                    prev = xT if l == 0 else gT[l - 1]
                    r0 = sched.q_r0[q]
                    for ib in range(sched.q_b0[q], sched.q_b1[q]):
                        ps = ppool.tile([128, 128], F32, tag="sup", name="ps_sup")
                        nc.tensor.matmul(
                            ps[:], prev[:, ib * 128:(ib + 1) * 128], w[:, l, :],
                            start=True, stop=True)
                        sup_sb = wpool.tile([128, 128], FP16, tag="sup_sb",
                                            name="sup_sb")
                        nc.scalar.activation(sup_sb[:], ps[:], Copy)
                        rows = min(128, shard - ib * 128)
                        lo = ib * 128 - r0
                        nc.sync.dma_start(
                            supq[l][q][lo:lo + rows, :], sup_sb[:rows, :])
                    nc.gpsimd.collective_compute(
                        "AllGather", mybir.AluOpType.bypass,
                        replica_groups=[list(range(ncores))],
                        ins=[supq[l][q].opt()], outs=[tblq[l][q].opt()])

                for q in range(Q):
                    emit_A(0, q)

                for l in range(3):
                    for gidx, gi in enumerate(sched.groups):
                        mts = [None] * Q
                        for q in range(Q):
                            ch0, C, binfo = gi.runs[q]
                            if C == 0:
                                continue
                            m = wpool.tile([128, C * 128], FP16, tag=f"msgs{q}",
                                           name=f"msgs{q}", bufs=2)
                            m3d = m[:].rearrange("p (c e) -> p c e", e=128)
                            for c0 in range(0, C, MAXG):
                                cn = min(MAXG, C - c0)
                                nc.gpsimd.dma_gather(
                                    out_ap=m3d[:, c0:c0 + cn, :],
                                    in_ap=tblq[l][q][:, :],
                                    idxs_ap=idxs[:, (ch0 + c0) * 8:
                                                 (ch0 + c0 + cn) * 8],
                                    num_idxs=cn * 128, num_idxs_reg=cn * 128,
                                    elem_size=128, queue_num=qstate[0])
                                qstate[0] = (qstate[0] + 1) % NQUEUES
                            mts[q] = m
                        for bi, b in enumerate(gi.blocks):
                            ps = ppool.tile([128, 128], F32, tag="agg",
                                            name="ps_agg")
                            total = sum(gi.runs[q][2][bi][2] for q in range(Q))
                            k = 0
                            for q in range(Q):
                                ch0, C, binfo = gi.runs[q]
                                _, loff, chn = binfo[bi]
                                if chn == 0:
                                    continue
                                m3 = mts[q][:].rearrange("p (c e) -> p c e",
                                                         e=128)
                                for i in range(chn):
                                    S = spool.tile([128, 128], FP16, tag="S",
                                                   name="S")
                                    nc.vector.tensor_scalar(
                                        S[:], iota[:],
                                        dstv[:, ch0 + loff + i:ch0 + loff + i + 1],
                                        valv[:, ch0 + loff + i:ch0 + loff + i + 1],
                                        iseq, mult)
                                    nc.tensor.matmul(
                                        ps[:], m3[:, loff + i, :], S[:],
                                        start=(k == 0), stop=(k == total - 1))
                                    k += 1
                            nc.scalar.activation(
                                gT[l][:, b * 128:(b + 1) * 128], ps[:], Relu,
                                bias=bl[:, l:l + 1])
                        if l < 2:
                            for q in range(Q):
                                if sched.q_done_g[q] == gidx:
                                    emit_A(l + 1, q)

                for ib in range(nb):
                    sl = slice(ib * 128, (ib + 1) * 128)
                    ps1 = ppool.tile([128, 128], F32, tag="fc1", name="ps_fc1",
                                     bufs=1)
                    for j in range(3):
                        nc.tensor.matmul(ps1[:], fw1[:, j, :], gT[j][:, sl],
                                         start=(j == 0), stop=(j == 2))
                    h1 = wpool.tile([128, 128], FP16, tag="h1", name="h1")
                    nc.scalar.activation(h1[:], ps1[:], Relu, bias=fb1[:, 0:1])
                    ps2 = ppool.tile([64, 128], F32, tag="fc2", name="ps_fc2",
                                     bufs=1)
                    nc.tensor.matmul(ps2[:], fw2[:], h1[:], start=True, stop=True)
                    h2 = wpool.tile([64, 128], FP16, tag="h2", name="h2")
                    nc.scalar.activation(h2[:], ps2[:], Relu, bias=fb2[:])
                    ps3 = ppool.tile([2, 128], F32, tag="fc3", name="ps_fc3",
                                     bufs=1)
                    nc.tensor.matmul(ps3[:], fw3[:], h2[:], start=True, stop=True)
                    nc.scalar.activation(outT[:, sl], ps3[:], Ident, bias=fb3[:])

                nc.sync.dma_start(out_d[:], outT[:])

    nc.compile()
    return nc


def _in_maps(inputs, sched, per_core, n_nodes, ncores):
    shard, npad = sched.shard, sched.nb * 128
    X = np.asarray(inputs["input_feature"], np.float32)
    xTs = []
    for c in range(ncores):
        xt = np.zeros((128, npad), np.float16)
        xt[:, :shard] = X[c * shard:(c + 1) * shard].T.astype(np.float16)
        xTs.append(xt)

    f16 = lambda a: np.ascontiguousarray(np.asarray(a, np.float32).astype(np.float16))
    f32 = lambda a: np.ascontiguousarray(np.asarray(a, np.float32))
    com = {
        "iota": np.ascontiguousarray(
            np.broadcast_to(np.arange(128, dtype=np.float16), (128, 128))),
        "w": np.stack([f16(inputs[k]) for k in ("W1", "W2", "W3")], axis=1),
        "b": np.stack([f32(inputs[k]) for k in ("b1", "b2", "b3")], axis=1),
        "fw1": np.ascontiguousarray(
            f16(inputs["fcW1"]).reshape(3, 128, 128).transpose(1, 0, 2)),
        "fb1": f32(inputs["fcb1"]).reshape(128, 1),
        "fw2": f16(inputs["fcW2"]),
        "fb2": f32(inputs["fcb2"]).reshape(64, 1),
        "fw3": f16(inputs["fcW3"]),
        "fb3": f32(inputs["fcb3"]).reshape(2, 1),
    }
    maps = []
    for c in range(ncores):
        m = dict(com)
        m["xT"] = xTs[c]
        m["idx"] = per_core[c].idx_sb
        m["dst"] = per_core[c].dst_sb
        m["val"] = per_core[c].val_sb
        maps.append(m)
    return maps


def _postprocess(results, sched, ncores):
    shard = sched.shard
    outs = [np.asarray(results[c]["out"], np.float32)[:, :shard].T
            for c in range(ncores)]
    return np.ascontiguousarray(np.concatenate(outs, axis=0))


_CACHE = {}


def _run(inputs, n_nodes, ncores, runner=None, enable_asserts=False, trace=False):
    row = np.asarray(inputs["adj_row"]).astype(np.int64)
    col = np.asarray(inputs["adj_col"]).astype(np.int64)
    vals = np.asarray(inputs["adj_vals"], np.float32)
    sched, per_core = _prepare(row, col, vals, n_nodes, ncores)
    nc = _build(sched, n_nodes, ncores, enable_asserts=enable_asserts)
    maps = _in_maps(inputs, sched, per_core, n_nodes, ncores)
    _CACHE["nc"], _CACHE["maps"], _CACHE["sched"] = nc, maps, sched
    if runner is None:
        res = run_bass_kernel_spmd(nc, maps, list(range(ncores)), trace=trace)
        results = res.results
        _CACHE["last_bench"] = res
    else:
        results = runner(nc, maps)
    return _postprocess(results, sched, ncores)


def kernel(**inputs):
    return _run(inputs, N_NODES, N_CORES)


# revision 11
# speedup vs baseline: 1361.5894x; 1.0960x over previous
"""GCN (3-layer graph conv + 3-layer MLP head) on 8 TRN2 NeuronCores.

Strategy (graph/1D-row parallel, per sharding hint):
  - Nodes row-sharded across 8 cores (6250 rows each). Per layer: local
    GEMM support = g_prev @ W on the node shard, AllGather the fp16
    support table, then each core aggregates its destination rows with
    dma_gather (neighbor rows) + one-hot scatter matmuls into f32 PSUM
    (S[e,dst]=val[e] built on DVE from a host-precomputed (dst,val)
    stream). Bias+ReLU+fp16 cast on ACT straight out of PSUM. Everything
    stays feature-major; the FC head runs the same way.
  - Layer 1 is reassociated (A@X)@W1 == A@(X@W1): X is a host input, so
    a pre-permuted replicated copy (xtbl, laid out exactly like the
    AllGather output) serves as the gather table — layer 1 needs no
    collective and its gathers start at t=0.
  - The remaining 2 AllGathers are chunked into Q=2 row-halves (each a
    Shared tile < 32768 rows, so int16 gather indices need no base
    split), and the next layer's local GEMM + AllGather half is emitted
    as soon as the destination blocks it needs are aggregated — the
    collective hides under the current layer's gather/aggregation.
  - The FC head is emitted per destination block inside layer 3's
    aggregation loop, so it overlaps the tail.
  - dma_gather calls round-robin 4 SWDGE queues with a 32KB/partition
    descriptor ring: the HBM-latency-bound random 256B reads get ~4x
    the concurrency of the single-queue default (the gather is the
    kernel's wall — ~460us/layer/core for 100k edges).

Numerics: fp16 storage / f32 PSUM accumulation -> ~2.6e-3 rel err vs
the f32 reference.
"""

import numpy as np

import concourse.bass as bass
import concourse.bacc as bacc
import concourse.mybir as mybir
import concourse.tile as tile
from concourse.bass_utils import run_bass_kernel_spmd

FP16 = mybir.dt.float16
F32 = mybir.dt.float32
I16 = mybir.dt.int16

N_NODES = 50000
N_CORES = 8
D = 128
NQ = 2        # AllGather row-chunks per layer (2: lowest chunk padding)
GSZ = 7       # dst blocks per gather group
MAXG = 6      # chunks per dma_gather call (ring capacity / packet limits)
SCRATCH = 32768
NQUEUES = 4
SPOOL_BUFS = 16
S_POOL_EVERY = 0   # every Nth S one-hot built on Pool instead of DVE (0=off; Pool elementwise is Q7 software — slow)


class _S:
    pass


def _prepare(row, col, vals, n_nodes, ncores, Q=NQ, gsz=GSZ):
    shard = n_nodes // ncores
    nb = (shard + 127) // 128
    ng = (nb + gsz - 1) // gsz

    qblocks = np.array_split(np.arange(nb), Q)
    q_b0 = [int(qb[0]) for qb in qblocks]
    q_b1 = [int(qb[-1]) + 1 for qb in qblocks]          # exclusive block end
    q_r0 = [b0 * 128 for b0 in q_b0]
    q_r1 = [min(b1 * 128, shard) for b1 in q_b1]
    rq = [r1 - r0 for r0, r1 in zip(q_r0, q_r1)]        # real rows per quadrant

    c = row // shard
    r = row % shard
    lb = r // 128
    dst = r % 128
    lbg = lb // gsz

    c_s = col // shard
    r_s = col % shard
    q_s = np.searchsorted(np.asarray(q_r0[1:]), r_s, side="right")
    loc = c_s * np.asarray(rq)[q_s] + (r_s - np.asarray(q_r0)[q_s])
    assert loc.max() < 32768

    order = np.lexsort((col, lb, q_s, lbg, c))
    c_o, lb_o, dst_o, q_o = c[order], lb[order], dst[order], q_s[order]
    loc_o, val_o = loc[order], vals[order]

    kk = ((c_o * ng + lb_o // gsz) * Q + q_o) * nb + lb_o
    cnt = np.zeros((ncores, nb, Q), np.int64)
    np.add.at(cnt, (c_o, lb_o, q_o), 1)
    CH = np.maximum.reduce(
        [(cnt[cc] + 127) // 128 for cc in range(ncores)])   # [nb, Q]
    for b in range(nb):
        if CH[b].sum() == 0:
            CH[b, 0] = 1

    groups = []
    tot_ch = 0
    for g0 in range(0, nb, gsz):
        blocks = list(range(g0, min(g0 + gsz, nb)))
        gi = _S()
        gi.runs = []      # per q: (ch0_abs, C, [(b, loff, chn), ...])
        for q in range(Q):
            loff = 0
            binfo = []
            ch0 = tot_ch
            for b in blocks:
                chn = int(CH[b, q])
                binfo.append((b, loff, chn))
                loff += chn
            tot_ch += loff
            gi.runs.append((ch0, loff, binfo))
        gi.blocks = blocks
        groups.append(gi)

    sched = _S()
    sched.shard, sched.nb, sched.ng, sched.Q, sched.gsz = shard, nb, ng, Q, gsz
    sched.tot_ch, sched.groups = tot_ch, groups
    sched.q_b0, sched.q_b1, sched.q_r0, sched.q_r1, sched.rq = q_b0, q_b1, q_r0, q_r1, rq
    # group index after which each quadrant's dst blocks are fully aggregated
    sched.q_done_g = [min(ng - 1, (b1 + gsz - 1) // gsz - 1) for b1 in q_b1]

    # chunk-order bounds per (core, group, q, block)
    per_core = []
    for cc in range(ncores):
        idx = np.zeros(tot_ch * 128, np.int16)
        dstv = np.zeros(tot_ch * 128, np.float32)
        valv = np.zeros(tot_ch * 128, np.float32)
        for gi in groups:
            for q in range(Q):
                ch0, C, binfo = gi.runs[q]
                for b, loff, chn in binfo:
                    if chn == 0:
                        continue
                    k2 = ((cc * ng + b // gsz) * Q + q) * nb + b
                    s, e = np.searchsorted(kk, [k2, k2 + 1])
                    n = e - s
                    o = (ch0 + loff) * 128
                    if n > 0:
                        idx[o:o+n] = loc_o[s:e].astype(np.int16)
                        dstv[o:o+n] = dst_o[s:e].astype(np.float32)
                        valv[o:o+n] = val_o[s:e].astype(np.float32)
        pc = _S()
        pc.idx_sb = np.tile(np.ascontiguousarray(idx.reshape(-1, 16).T), (8, 1))
        pc.dst_sb = np.ascontiguousarray(dstv.reshape(tot_ch, 128).T)
        pc.val_sb = np.ascontiguousarray(valv.reshape(tot_ch, 128).T)
        per_core.append(pc)
    return sched, per_core


def _build(sched, n_nodes, ncores, enable_asserts=False, passes=1):
    nb, shard, tot_ch = sched.nb, sched.shard, sched.tot_ch
    Q, rq = sched.Q, sched.rq
    npad = nb * 128

    nc = bacc.Bacc(
        "TRN2",
        target_bir_lowering=False,
        debug=False,
        enable_asserts=enable_asserts,
        num_devices=ncores,
        dynamic_dma_scratch_size=SCRATCH,
        num_swdge_queues=NQUEUES,
    )

    xtbl_d = nc.declare_dram_parameter("xtbl", [n_nodes, 128], FP16, isOutput=False)
    idx_d = nc.declare_dram_parameter("idx", [128, tot_ch * 8], I16, isOutput=False)
    dst_d = nc.declare_dram_parameter("dst", [128, tot_ch], F32, isOutput=False)
    val_d = nc.declare_dram_parameter("val", [128, tot_ch], F32, isOutput=False)
    iota_d = nc.declare_dram_parameter("iota", [128, 128], FP16, isOutput=False)
    w_d = nc.declare_dram_parameter("w", [128, 3, 128], FP16, isOutput=False)
    b_d = nc.declare_dram_parameter("b", [128, 3], F32, isOutput=False)
    fw1_d = nc.declare_dram_parameter("fw1", [128, 3, 128], FP16, isOutput=False)
    fb1_d = nc.declare_dram_parameter("fb1", [128, 1], F32, isOutput=False)
    fw2_d = nc.declare_dram_parameter("fw2", [128, 64], FP16, isOutput=False)
    fb2_d = nc.declare_dram_parameter("fb2", [64, 1], F32, isOutput=False)
    fw3_d = nc.declare_dram_parameter("fw3", [64, 2], FP16, isOutput=False)
    fb3_d = nc.declare_dram_parameter("fb3", [2, 1], F32, isOutput=False)
    out_d = nc.declare_dram_parameter("out", [2, npad], F32, isOutput=True)

    Relu = mybir.ActivationFunctionType.Relu
    Copy = mybir.ActivationFunctionType.Copy
    Ident = mybir.ActivationFunctionType.Identity
    iseq = mybir.AluOpType.is_equal
    mult = mybir.AluOpType.mult

    qstate = [0]

    with tile.TileContext(nc) as tc:
        with (
            tc.tile_pool(name="const", bufs=1) as cpool,
            tc.tile_pool(name="dram", bufs=1, space="DRAM") as dpool,
            tc.tile_pool(name="work", bufs=3) as wpool,
            tc.tile_pool(name="sbuild", bufs=SPOOL_BUFS) as spool,
            tc.tile_pool(name="psum", bufs=2, space="PSUM") as ppool,
        ):
            def load(d, shape, dtype, name):
                t = cpool.tile(list(shape), dtype, name=name)
                nc.sync.dma_start(t[:], d[:])
                return t

            idxs = load(idx_d, [128, tot_ch * 8], I16, "idxs")
            dstv = load(dst_d, [128, tot_ch], F32, "dstv")
            valv = load(val_d, [128, tot_ch], F32, "valv")
            iota = load(iota_d, [128, 128], FP16, "iota")
            w = load(w_d, [128, 3, 128], FP16, "w")
            bl = load(b_d, [128, 3], F32, "bl")
            fw1 = load(fw1_d, [128, 3, 128], FP16, "fw1")
            fb1 = load(fb1_d, [128, 1], F32, "fb1")
            fw2 = load(fw2_d, [128, 64], FP16, "fw2")
            fb2 = load(fb2_d, [64, 1], F32, "fb2")
            fw3 = load(fw3_d, [64, 2], FP16, "fw3")
            fb3 = load(fb3_d, [2, 1], F32, "fb3")

            for _pass in range(passes):
                gT = [cpool.tile([128, npad], FP16, name=f"gT{l}", tag=f"gT{l}")
                      for l in range(3)]
                outT = cpool.tile([2, npad], F32, tag="outT", name="outT")
                qoff = np.cumsum([0] + [ncores * r for r in rq]).tolist()
                ltbl = lambda l, q: (xtbl_d[qoff[q]:qoff[q] + ncores * rq[q], :]
                                     if l == 0 else tblq[l][q][:, :])
                supq = [[dpool.tile([rq[q], 128], FP16,
                                    name=f"sup{l}q{q}_{_pass}",
                                    tag=f"sup{l}q{q}_{_pass}")
                         for q in range(Q)] for l in range(1, 3)]
                tblq = [[dpool.tile([ncores * rq[q], 128], FP16,
                                    addr_space="Shared",
                                    name=f"tbl{l}q{q}_{_pass}",
                                    tag=f"tbl{l}q{q}_{_pass}")
                         for q in range(Q)] for l in range(1, 3)]
                supq = dict(zip((1, 2), supq))
                tblq = dict(zip((1, 2), tblq))

                def emit_A(l, q):
                    """sup GEMMs for quadrant q of layer l, then AllGather it."""
                    prev = gT[l - 1]
                    r0 = sched.q_r0[q]
                    for ib in range(sched.q_b0[q], sched.q_b1[q]):
                        ps = ppool.tile([128, 128], F32, tag="sup", name="ps_sup")
                        nc.tensor.matmul(
                            ps[:], prev[:, ib * 128:(ib + 1) * 128], w[:, l, :],
                            start=True, stop=True)
                        sup_sb = wpool.tile([128, 128], FP16, tag="sup_sb",
                                            name="sup_sb")
                        nc.scalar.activation(sup_sb[:], ps[:], Copy)
                        rows = min(128, shard - ib * 128)
                        lo = ib * 128 - r0
                        nc.sync.dma_start(
                            supq[l][q][lo:lo + rows, :], sup_sb[:rows, :])
                    nc.gpsimd.collective_compute(
                        "AllGather", mybir.AluOpType.bypass,
                        replica_groups=[list(range(ncores))],
                        ins=[supq[l][q].opt()], outs=[tblq[l][q].opt()])

                def emit_fc(ib):
                    sl = slice(ib * 128, (ib + 1) * 128)
                    ps1 = ppool.tile([128, 128], F32, tag="fc1", name="ps_fc1",
                                     bufs=1)
                    for j in range(3):
                        nc.tensor.matmul(ps1[:], fw1[:, j, :], gT[j][:, sl],
                                         start=(j == 0), stop=(j == 2))
                    h1 = wpool.tile([128, 128], FP16, tag="h1", name="h1")
                    nc.scalar.activation(h1[:], ps1[:], Relu, bias=fb1[:, 0:1])
                    ps2 = ppool.tile([64, 128], F32, tag="fc2", name="ps_fc2",
                                     bufs=1)
                    nc.tensor.matmul(ps2[:], fw2[:], h1[:], start=True, stop=True)
                    h2 = wpool.tile([64, 128], FP16, tag="h2", name="h2")
                    nc.scalar.activation(h2[:], ps2[:], Relu, bias=fb2[:])
                    ps3 = ppool.tile([2, 128], F32, tag="fc3", name="ps_fc3",
                                     bufs=1)
                    nc.tensor.matmul(ps3[:], fw3[:], h2[:], start=True, stop=True)
                    nc.scalar.activation(outT[:, sl], ps3[:], Ident, bias=fb3[:])

                for l in range(3):
                    for gidx, gi in enumerate(sched.groups):
                        mts = [None] * Q
                        for q in range(Q):
                            ch0, C, binfo = gi.runs[q]
                            if C == 0:
                                continue
                            m = wpool.tile([128, C * 128], FP16, tag=f"msgs{q}",
                                           name=f"msgs{q}", bufs=2)
                            m3d = m[:].rearrange("p (c e) -> p c e", e=128)
                            for c0 in range(0, C, MAXG):
                                cn = min(MAXG, C - c0)
                                nc.gpsimd.dma_gather(
                                    out_ap=m3d[:, c0:c0 + cn, :],
                                    in_ap=ltbl(l, q),
                                    idxs_ap=idxs[:, (ch0 + c0) * 8:
                                                 (ch0 + c0 + cn) * 8],
                                    num_idxs=cn * 128, num_idxs_reg=cn * 128,
                                    elem_size=128, queue_num=qstate[0])
                                qstate[0] = (qstate[0] + 1) % NQUEUES
                            mts[q] = m
                        for bi, b in enumerate(gi.blocks):
                            ps = ppool.tile([128, 128], F32, tag="agg",
                                            name="ps_agg")
                            total = sum(gi.runs[q][2][bi][2] for q in range(Q))
                            k = 0
                            for q in range(Q):
                                ch0, C, binfo = gi.runs[q]
                                _, loff, chn = binfo[bi]
                                if chn == 0:
                                    continue
                                m3 = mts[q][:].rearrange("p (c e) -> p c e",
                                                         e=128)
                                for i in range(chn):
                                    S = spool.tile([128, 128], FP16, tag="S",
                                                   name="S")
                                    seng = (nc.gpsimd if S_POOL_EVERY and
                                            (ch0 + loff + i) % S_POOL_EVERY == 0
                                            else nc.vector)
                                    seng.tensor_scalar(
                                        S[:], iota[:],
                                        dstv[:, ch0 + loff + i:ch0 + loff + i + 1],
                                        valv[:, ch0 + loff + i:ch0 + loff + i + 1],
                                        iseq, mult)
                                    nc.tensor.matmul(
                                        ps[:], m3[:, loff + i, :], S[:],
                                        start=(k == 0), stop=(k == total - 1))
                                    k += 1
                            if l == 0:
                                # layer 1 reassociated: (A.X) @ W1
                                ax = wpool.tile([128, 128], FP16, tag="ax",
                                                name="ax")
                                nc.scalar.activation(ax[:], ps[:], Copy)
                                ps0 = ppool.tile([128, 128], F32, tag="sup",
                                                 name="ps_axw")
                                nc.tensor.matmul(ps0[:], w[:, 0, :], ax[:],
                                                 start=True, stop=True)
                                ps = ps0
                            nc.scalar.activation(
                                gT[l][:, b * 128:(b + 1) * 128], ps[:], Relu,
                                bias=bl[:, l:l + 1])
                            if l == 2:
                                emit_fc(b)
                        if l < 2:
                            for q in range(Q):
                                if sched.q_done_g[q] == gidx:
                                    emit_A(l + 1, q)

                nc.sync.dma_start(out_d[:], outT[:])

    nc.compile()
    return nc


def _in_maps(inputs, sched, per_core, n_nodes, ncores):
    shard = sched.shard
    X = np.asarray(inputs["input_feature"], np.float32).astype(np.float16)
    # X permuted into the AllGather-output layout: q-major, core-major
    xtbl = np.concatenate(
        [X[c * shard + sched.q_r0[q]:c * shard + sched.q_r1[q]]
         for q in range(sched.Q) for c in range(ncores)], axis=0)
    xtbl = np.ascontiguousarray(xtbl)

    f16 = lambda a: np.ascontiguousarray(np.asarray(a, np.float32).astype(np.float16))
    f32 = lambda a: np.ascontiguousarray(np.asarray(a, np.float32))
    com = {
        "iota": np.ascontiguousarray(
            np.broadcast_to(np.arange(128, dtype=np.float16), (128, 128))),
        "w": np.stack([f16(inputs[k]) for k in ("W1", "W2", "W3")], axis=1),
        "b": np.stack([f32(inputs[k]) for k in ("b1", "b2", "b3")], axis=1),
        "fw1": np.ascontiguousarray(
            f16(inputs["fcW1"]).reshape(3, 128, 128).transpose(1, 0, 2)),
        "fb1": f32(inputs["fcb1"]).reshape(128, 1),
        "fw2": f16(inputs["fcW2"]),
        "fb2": f32(inputs["fcb2"]).reshape(64, 1),
        "fw3": f16(inputs["fcW3"]),
        "fb3": f32(inputs["fcb3"]).reshape(2, 1),
    }
    maps = []
    for c in range(ncores):
        m = dict(com)
        m["xtbl"] = xtbl
        m["idx"] = per_core[c].idx_sb
        m["dst"] = per_core[c].dst_sb
        m["val"] = per_core[c].val_sb
        maps.append(m)
    return maps


def _postprocess(results, sched, ncores):
    shard = sched.shard
    outs = [np.asarray(results[c]["out"], np.float32)[:, :shard].T
            for c in range(ncores)]
    return np.ascontiguousarray(np.concatenate(outs, axis=0))


_CACHE = {}


def _run(inputs, n_nodes, ncores, runner=None, enable_asserts=False, trace=False):
    row = np.asarray(inputs["adj_row"]).astype(np.int64)
    col = np.asarray(inputs["adj_col"]).astype(np.int64)
    vals = np.asarray(inputs["adj_vals"], np.float32)
    sched, per_core = _prepare(row, col, vals, n_nodes, ncores)
    nc = _build(sched, n_nodes, ncores, enable_asserts=enable_asserts)
    maps = _in_maps(inputs, sched, per_core, n_nodes, ncores)
    _CACHE["nc"], _CACHE["maps"], _CACHE["sched"] = nc, maps, sched
    if runner is None:
        res = run_bass_kernel_spmd(nc, maps, list(range(ncores)), trace=trace)
        results = res.results
        _CACHE["last_bench"] = res
    else:
        results = runner(nc, maps)
    return _postprocess(results, sched, ncores)


def kernel(**inputs):
    return _run(inputs, N_NODES, N_CORES)


# revision 14
# speedup vs baseline: 1413.2249x; 1.0379x over previous
"""GCN (3-layer graph conv + 3-layer MLP head) on 8 TRN2 NeuronCores.

Strategy (graph/1D-row parallel, per sharding hint):
  - Nodes row-sharded across 8 cores (6250 rows each). Per layer: local
    GEMM support = g_prev @ W on the node shard, AllGather the fp16
    support table, then each core aggregates its destination rows with
    dma_gather (neighbor rows) + one-hot scatter matmuls into f32 PSUM
    (S[e,dst]=val[e] built on DVE from a host-precomputed (dst,val)
    stream). Bias+ReLU+fp16 cast on ACT straight out of PSUM. Everything
    stays feature-major; the FC head runs the same way.
  - Layer 1 is reassociated (A@X)@W1 == A@(X@W1): X is a host input, so
    a pre-permuted replicated copy (xtbl, laid out exactly like the
    AllGather output) serves as the gather table — layer 1 needs no
    collective and its gathers start at t=0.
  - The remaining 2 AllGathers are chunked into Q=2 row-halves (each a
    Shared tile < 32768 rows, so int16 gather indices need no base
    split), and the next layer's local GEMM + AllGather half is emitted
    as soon as the destination blocks it needs are aggregated — the
    collective hides under the current layer's gather/aggregation.
  - The FC head is emitted per destination block inside layer 3's
    aggregation loop, so it overlaps the tail.
  - dma_gather calls round-robin 4 SWDGE queues with a 32KB/partition
    descriptor ring: the HBM-latency-bound random 256B reads get ~4x
    the concurrency of the single-queue default (the gather is the
    kernel's wall — ~460us/layer/core for 100k edges).

Numerics: fp16 storage / f32 PSUM accumulation -> ~2.6e-3 rel err vs
the f32 reference.
"""

import numpy as np

import concourse.bass as bass
import concourse.bacc as bacc
import concourse.mybir as mybir
import concourse.tile as tile
from concourse.bass_utils import run_bass_kernel_spmd

FP16 = mybir.dt.float16
F32 = mybir.dt.float32
I16 = mybir.dt.int16

N_NODES = 50000
N_CORES = 8
D = 128
NQ = 2        # AllGather row-chunks per layer (2: lowest chunk padding)
GSZ = 4       # dst blocks per gather group
MAXG = 6      # chunks per dma_gather call (ring capacity / packet limits)
SP = True     # single_packet (must be False for MAXG>8: >64 descs/engine)
SCRATCH = 32768
NQUEUES = 4
SPOOL_BUFS = 24
MSGS_BUFS = 3
S_POOL_EVERY = 0   # every Nth S one-hot built on Pool instead of DVE (0=off; Pool elementwise is Q7 software — slow)


class _S:
    pass


def _prepare(row, col, vals, n_nodes, ncores, Q=NQ, gsz=GSZ):
    shard = n_nodes // ncores
    nb = (shard + 127) // 128
    ng = (nb + gsz - 1) // gsz

    qblocks = np.array_split(np.arange(nb), Q)
    q_b0 = [int(qb[0]) for qb in qblocks]
    q_b1 = [int(qb[-1]) + 1 for qb in qblocks]          # exclusive block end
    q_r0 = [b0 * 128 for b0 in q_b0]
    q_r1 = [min(b1 * 128, shard) for b1 in q_b1]
    rq = [r1 - r0 for r0, r1 in zip(q_r0, q_r1)]        # real rows per quadrant

    c = row // shard
    r = row % shard
    lb = r // 128
    dst = r % 128
    lbg = lb // gsz

    c_s = col // shard
    r_s = col % shard
    q_s = np.searchsorted(np.asarray(q_r0[1:]), r_s, side="right")
    loc = c_s * np.asarray(rq)[q_s] + (r_s - np.asarray(q_r0)[q_s])
    assert loc.max() < 32768

    order = np.lexsort((col, lb, q_s, lbg, c))
    c_o, lb_o, dst_o, q_o = c[order], lb[order], dst[order], q_s[order]
    loc_o, val_o = loc[order], vals[order]

    kk = ((c_o * ng + lb_o // gsz) * Q + q_o) * nb + lb_o
    cnt = np.zeros((ncores, nb, Q), np.int64)
    np.add.at(cnt, (c_o, lb_o, q_o), 1)
    CH = np.maximum.reduce(
        [(cnt[cc] + 127) // 128 for cc in range(ncores)])   # [nb, Q]
    for b in range(nb):
        if CH[b].sum() == 0:
            CH[b, 0] = 1

    groups = []
    tot_ch = 0
    for g0 in range(0, nb, gsz):
        blocks = list(range(g0, min(g0 + gsz, nb)))
        gi = _S()
        gi.runs = []      # per q: (ch0_abs, C, [(b, loff, chn), ...])
        for q in range(Q):
            loff = 0
            binfo = []
            ch0 = tot_ch
            for b in blocks:
                chn = int(CH[b, q])
                binfo.append((b, loff, chn))
                loff += chn
            tot_ch += loff
            gi.runs.append((ch0, loff, binfo))
        gi.blocks = blocks
        groups.append(gi)

    sched = _S()
    sched.shard, sched.nb, sched.ng, sched.Q, sched.gsz = shard, nb, ng, Q, gsz
    sched.tot_ch, sched.groups = tot_ch, groups
    sched.q_b0, sched.q_b1, sched.q_r0, sched.q_r1, sched.rq = q_b0, q_b1, q_r0, q_r1, rq
    # group index after which each quadrant's dst blocks are fully aggregated
    sched.q_done_g = [min(ng - 1, (b1 + gsz - 1) // gsz - 1) for b1 in q_b1]

    # chunk-order bounds per (core, group, q, block)
    per_core = []
    for cc in range(ncores):
        idx = np.zeros(tot_ch * 128, np.int16)
        dstv = np.zeros(tot_ch * 128, np.float32)
        valv = np.zeros(tot_ch * 128, np.float32)
        for gi in groups:
            for q in range(Q):
                ch0, C, binfo = gi.runs[q]
                for b, loff, chn in binfo:
                    if chn == 0:
                        continue
                    k2 = ((cc * ng + b // gsz) * Q + q) * nb + b
                    s, e = np.searchsorted(kk, [k2, k2 + 1])
                    n = e - s
                    o = (ch0 + loff) * 128
                    if n > 0:
                        idx[o:o+n] = loc_o[s:e].astype(np.int16)
                        dstv[o:o+n] = dst_o[s:e].astype(np.float32)
                        valv[o:o+n] = val_o[s:e].astype(np.float32)
        pc = _S()
        pc.idx_sb = np.tile(np.ascontiguousarray(idx.reshape(-1, 16).T), (8, 1))
        pc.dst_sb = np.ascontiguousarray(dstv.reshape(tot_ch, 128).T)
        pc.val_sb = np.ascontiguousarray(valv.reshape(tot_ch, 128).T)
        per_core.append(pc)
    return sched, per_core


def _build(sched, n_nodes, ncores, enable_asserts=False, passes=1):
    nb, shard, tot_ch = sched.nb, sched.shard, sched.tot_ch
    Q, rq = sched.Q, sched.rq
    npad = nb * 128

    nc = bacc.Bacc(
        "TRN2",
        target_bir_lowering=False,
        debug=False,
        enable_asserts=enable_asserts,
        num_devices=ncores,
        dynamic_dma_scratch_size=SCRATCH,
        num_swdge_queues=NQUEUES,
    )

    xtbl_d = nc.declare_dram_parameter("xtbl", [n_nodes, 128], FP16, isOutput=False)
    idx_d = nc.declare_dram_parameter("idx", [128, tot_ch * 8], I16, isOutput=False)
    dst_d = nc.declare_dram_parameter("dst", [128, tot_ch], F32, isOutput=False)
    val_d = nc.declare_dram_parameter("val", [128, tot_ch], F32, isOutput=False)
    iota_d = nc.declare_dram_parameter("iota", [128, 128], FP16, isOutput=False)
    w_d = nc.declare_dram_parameter("w", [128, 3, 128], FP16, isOutput=False)
    b_d = nc.declare_dram_parameter("b", [128, 3], F32, isOutput=False)
    fw1_d = nc.declare_dram_parameter("fw1", [128, 3, 128], FP16, isOutput=False)
    fb1_d = nc.declare_dram_parameter("fb1", [128, 1], F32, isOutput=False)
    fw2_d = nc.declare_dram_parameter("fw2", [128, 64], FP16, isOutput=False)
    fb2_d = nc.declare_dram_parameter("fb2", [64, 1], F32, isOutput=False)
    fw3_d = nc.declare_dram_parameter("fw3", [64, 2], FP16, isOutput=False)
    fb3_d = nc.declare_dram_parameter("fb3", [2, 1], F32, isOutput=False)
    out_d = nc.declare_dram_parameter("out", [2, npad], F32, isOutput=True)

    Relu = mybir.ActivationFunctionType.Relu
    Copy = mybir.ActivationFunctionType.Copy
    Ident = mybir.ActivationFunctionType.Identity
    iseq = mybir.AluOpType.is_equal
    mult = mybir.AluOpType.mult

    qstate = [0]

    with tile.TileContext(nc) as tc:
        with (
            tc.tile_pool(name="const", bufs=1) as cpool,
            tc.tile_pool(name="dram", bufs=1, space="DRAM") as dpool,
            tc.tile_pool(name="work", bufs=3) as wpool,
            tc.tile_pool(name="sbuild", bufs=SPOOL_BUFS) as spool,
            tc.tile_pool(name="psum", bufs=2, space="PSUM") as ppool,
        ):
            def load(d, shape, dtype, name):
                t = cpool.tile(list(shape), dtype, name=name)
                nc.sync.dma_start(t[:], d[:])
                return t

            idxs = load(idx_d, [128, tot_ch * 8], I16, "idxs")
            dstv = load(dst_d, [128, tot_ch], F32, "dstv")
            valv = load(val_d, [128, tot_ch], F32, "valv")
            iota = load(iota_d, [128, 128], FP16, "iota")
            w = load(w_d, [128, 3, 128], FP16, "w")
            bl = load(b_d, [128, 3], F32, "bl")
            fw1 = load(fw1_d, [128, 3, 128], FP16, "fw1")
            fb1 = load(fb1_d, [128, 1], F32, "fb1")
            fw2 = load(fw2_d, [128, 64], FP16, "fw2")
            fb2 = load(fb2_d, [64, 1], F32, "fb2")
            fw3 = load(fw3_d, [64, 2], FP16, "fw3")
            fb3 = load(fb3_d, [2, 1], F32, "fb3")

            for _pass in range(passes):
                gT = [cpool.tile([128, npad], FP16, name=f"gT{l}", tag=f"gT{l}")
                      for l in range(3)]
                outT = cpool.tile([2, npad], F32, tag="outT", name="outT")
                qoff = np.cumsum([0] + [ncores * r for r in rq]).tolist()
                ltbl = lambda l, q: (xtbl_d[qoff[q]:qoff[q] + ncores * rq[q], :]
                                     if l == 0 else tblq[l][q][:, :])
                supq = [[dpool.tile([rq[q], 128], FP16,
                                    name=f"sup{l}q{q}_{_pass}",
                                    tag=f"sup{l}q{q}_{_pass}")
                         for q in range(Q)] for l in range(1, 3)]
                tblq = [[dpool.tile([ncores * rq[q], 128], FP16,
                                    addr_space="Shared",
                                    name=f"tbl{l}q{q}_{_pass}",
                                    tag=f"tbl{l}q{q}_{_pass}")
                         for q in range(Q)] for l in range(1, 3)]
                supq = dict(zip((1, 2), supq))
                tblq = dict(zip((1, 2), tblq))

                def emit_A(l, q):
                    """sup GEMMs for quadrant q of layer l, then AllGather it."""
                    prev = gT[l - 1]
                    r0 = sched.q_r0[q]
                    for ib in range(sched.q_b0[q], sched.q_b1[q]):
                        ps = ppool.tile([128, 128], F32, tag="sup", name="ps_sup")
                        nc.tensor.matmul(
                            ps[:], prev[:, ib * 128:(ib + 1) * 128], w[:, l, :],
                            start=True, stop=True)
                        sup_sb = wpool.tile([128, 128], FP16, tag="sup_sb",
                                            name="sup_sb")
                        nc.scalar.activation(sup_sb[:], ps[:], Copy)
                        rows = min(128, shard - ib * 128)
                        lo = ib * 128 - r0
                        nc.sync.dma_start(
                            supq[l][q][lo:lo + rows, :], sup_sb[:rows, :])
                    nc.gpsimd.collective_compute(
                        "AllGather", mybir.AluOpType.bypass,
                        replica_groups=[list(range(ncores))],
                        ins=[supq[l][q].opt()], outs=[tblq[l][q].opt()])

                def emit_fc(ib):
                    sl = slice(ib * 128, (ib + 1) * 128)
                    ps1 = ppool.tile([128, 128], F32, tag="fc1", name="ps_fc1",
                                     bufs=1)
                    for j in range(3):
                        nc.tensor.matmul(ps1[:], fw1[:, j, :], gT[j][:, sl],
                                         start=(j == 0), stop=(j == 2))
                    h1 = wpool.tile([128, 128], FP16, tag="h1", name="h1")
                    nc.scalar.activation(h1[:], ps1[:], Relu, bias=fb1[:, 0:1])
                    ps2 = ppool.tile([64, 128], F32, tag="fc2", name="ps_fc2",
                                     bufs=1)
                    nc.tensor.matmul(ps2[:], fw2[:], h1[:], start=True, stop=True)
                    h2 = wpool.tile([64, 128], FP16, tag="h2", name="h2")
                    nc.scalar.activation(h2[:], ps2[:], Relu, bias=fb2[:])
                    ps3 = ppool.tile([2, 128], F32, tag="fc3", name="ps_fc3",
                                     bufs=1)
                    nc.tensor.matmul(ps3[:], fw3[:], h2[:], start=True, stop=True)
                    nc.scalar.activation(outT[:, sl], ps3[:], Ident, bias=fb3[:])

                for l in range(3):
                    for gidx, gi in enumerate(sched.groups):
                        mts = [None] * Q
                        for q in range(Q):
                            ch0, C, binfo = gi.runs[q]
                            if C == 0:
                                continue
                            m = wpool.tile([128, C * 128], FP16, tag=f"msgs{q}",
                                           name=f"msgs{q}", bufs=MSGS_BUFS)
                            m3d = m[:].rearrange("p (c e) -> p c e", e=128)
                            for c0 in range(0, C, MAXG):
                                cn = min(MAXG, C - c0)
                                nc.gpsimd.dma_gather(
                                    out_ap=m3d[:, c0:c0 + cn, :],
                                    in_ap=ltbl(l, q),
                                    idxs_ap=idxs[:, (ch0 + c0) * 8:
                                                 (ch0 + c0 + cn) * 8],
                                    num_idxs=cn * 128, num_idxs_reg=cn * 128,
                                    elem_size=128, queue_num=qstate[0],
                                    single_packet=SP)
                                qstate[0] = (qstate[0] + 1) % NQUEUES
                            mts[q] = m
                        for bi, b in enumerate(gi.blocks):
                            ps = ppool.tile([128, 128], F32, tag="agg",
                                            name="ps_agg")
                            total = sum(gi.runs[q][2][bi][2] for q in range(Q))
                            k = 0
                            for q in range(Q):
                                ch0, C, binfo = gi.runs[q]
                                _, loff, chn = binfo[bi]
                                if chn == 0:
                                    continue
                                m3 = mts[q][:].rearrange("p (c e) -> p c e",
                                                         e=128)
                                for i in range(chn):
                                    S = spool.tile([128, 128], FP16, tag="S",
                                                   name="S")
                                    seng = (nc.gpsimd if S_POOL_EVERY and
                                            (ch0 + loff + i) % S_POOL_EVERY == 0
                                            else nc.vector)
                                    seng.tensor_scalar(
                                        S[:], iota[:],
                                        dstv[:, ch0 + loff + i:ch0 + loff + i + 1],
                                        valv[:, ch0 + loff + i:ch0 + loff + i + 1],
                                        iseq, mult)
                                    nc.tensor.matmul(
                                        ps[:], m3[:, loff + i, :], S[:],
                                        start=(k == 0), stop=(k == total - 1))
                                    k += 1
                            if l == 0:
                                # layer 1 reassociated: (A.X) @ W1
                                ax = wpool.tile([128, 128], FP16, tag="ax",
                                                name="ax")
                                nc.scalar.activation(ax[:], ps[:], Copy)
                                ps0 = ppool.tile([128, 128], F32, tag="sup",
                                                 name="ps_axw")
                                nc.tensor.matmul(ps0[:], w[:, 0, :], ax[:],
                                                 start=True, stop=True)
                                ps = ps0
                            nc.scalar.activation(
                                gT[l][:, b * 128:(b + 1) * 128], ps[:], Relu,
                                bias=bl[:, l:l + 1])
                            if l == 2:
                                emit_fc(b)
                        if l < 2:
                            for q in range(Q):
                                if sched.q_done_g[q] == gidx:
                                    emit_A(l + 1, q)

                nc.sync.dma_start(out_d[:], outT[:])

    nc.compile()
    return nc


def _in_maps(inputs, sched, per_core, n_nodes, ncores):
    shard = sched.shard
    X = np.asarray(inputs["input_feature"], np.float32).astype(np.float16)
    # X permuted into the AllGather-output layout: q-major, core-major
    xtbl = np.concatenate(
        [X[c * shard + sched.q_r0[q]:c * shard + sched.q_r1[q]]
         for q in range(sched.Q) for c in range(ncores)], axis=0)
    xtbl = np.ascontiguousarray(xtbl)

    f16 = lambda a: np.ascontiguousarray(np.asarray(a, np.float32).astype(np.float16))
    f32 = lambda a: np.ascontiguousarray(np.asarray(a, np.float32))
    com = {
        "iota": np.ascontiguousarray(
            np.broadcast_to(np.arange(128, dtype=np.float16), (128, 128))),
        "w": np.stack([f16(inputs[k]) for k in ("W1", "W2", "W3")], axis=1),
        "b": np.stack([f32(inputs[k]) for k in ("b1", "b2", "b3")], axis=1),
        "fw1": np.ascontiguousarray(
            f16(inputs["fcW1"]).reshape(3, 128, 128).transpose(1, 0, 2)),
        "fb1": f32(inputs["fcb1"]).reshape(128, 1),
        "fw2": f16(inputs["fcW2"]),
        "fb2": f32(inputs["fcb2"]).reshape(64, 1),
        "fw3": f16(inputs["fcW3"]),
        "fb3": f32(inputs["fcb3"]).reshape(2, 1),
    }
    maps = []
    for c in range(ncores):
        m = dict(com)
        m["xtbl"] = xtbl
        m["idx"] = per_core[c].idx_sb
        m["dst"] = per_core[c].dst_sb
        m["val"] = per_core[c].val_sb
        maps.append(m)
    return maps


def _postprocess(results, sched, ncores):
    shard = sched.shard
    outs = [np.asarray(results[c]["out"], np.float32)[:, :shard].T
            for c in range(ncores)]
    return np.ascontiguousarray(np.concatenate(outs, axis=0))


_CACHE = {}


def _run(inputs, n_nodes, ncores, runner=None, enable_asserts=False, trace=False):
    row = np.asarray(inputs["adj_row"]).astype(np.int64)
    col = np.asarray(inputs["adj_col"]).astype(np.int64)
    vals = np.asarray(inputs["adj_vals"], np.float32)
    sched, per_core = _prepare(row, col, vals, n_nodes, ncores)
    nc = _build(sched, n_nodes, ncores, enable_asserts=enable_asserts)
    maps = _in_maps(inputs, sched, per_core, n_nodes, ncores)
    _CACHE["nc"], _CACHE["maps"], _CACHE["sched"] = nc, maps, sched
    if runner is None:
        res = run_bass_kernel_spmd(nc, maps, list(range(ncores)), trace=trace)
        results = res.results
        _CACHE["last_bench"] = res
    else:
        results = runner(nc, maps)
    return _postprocess(results, sched, ncores)


def kernel(**inputs):
    return _run(inputs, N_NODES, N_CORES)


# revision 15
# speedup vs baseline: 1421.5802x; 1.0059x over previous
"""GCN (3-layer graph conv + 3-layer MLP head) on 8 TRN2 NeuronCores.

Strategy (graph/1D-row parallel, per sharding hint):
  - Nodes row-sharded across 8 cores (6250 rows each). Per layer: local
    GEMM support = g_prev @ W on the node shard, AllGather the fp16
    support table, then each core aggregates its destination rows with
    dma_gather (neighbor rows) + one-hot scatter matmuls into f32 PSUM
    (S[e,dst]=val[e] built on DVE from a host-precomputed (dst,val)
    stream). Bias+ReLU+fp16 cast on ACT straight out of PSUM. Everything
    stays feature-major; the FC head runs the same way.
  - Layer 1 is reassociated (A@X)@W1 == A@(X@W1): X is a host input, so
    a pre-permuted replicated copy (xtbl, laid out exactly like the
    AllGather output) serves as the gather table — layer 1 needs no
    collective and its gathers start at t=0.
  - The remaining 2 AllGathers are chunked into Q=2 row-halves (each a
    Shared tile < 32768 rows, so int16 gather indices need no base
    split), and the next layer's local GEMM + AllGather half is emitted
    as soon as the destination blocks it needs are aggregated — the
    collective hides under the current layer's gather/aggregation.
  - The FC head is emitted per destination block inside layer 3's
    aggregation loop, so it overlaps the tail.
  - dma_gather calls round-robin 4 SWDGE queues with a 32KB/partition
    descriptor ring: the HBM-latency-bound random 256B reads get ~4x
    the concurrency of the single-queue default (the gather is the
    kernel's wall — ~460us/layer/core for 100k edges).

Numerics: fp16 storage / f32 PSUM accumulation -> ~2.6e-3 rel err vs
the f32 reference.
"""

import numpy as np

import concourse.bass as bass
import concourse.bacc as bacc
import concourse.mybir as mybir
import concourse.tile as tile
from concourse.bass_utils import run_bass_kernel_spmd

FP16 = mybir.dt.float16
F32 = mybir.dt.float32
I16 = mybir.dt.int16

N_NODES = 50000
N_CORES = 8
D = 128
NQ = 2        # AllGather row-chunks per layer (2: lowest chunk padding)
GSZ = 4       # dst blocks per gather group
MAXG = 6      # chunks per dma_gather call (ring capacity / packet limits)
SP = True     # single_packet (must be False for MAXG>8: >64 descs/engine)
SCRATCH = 32768
NQUEUES = 4
SPOOL_BUFS = 24
MSGS_BUFS = 3
AG_DELAY = 2  # groups between sup emission and its collective (stall hiding)
S_POOL_EVERY = 0   # every Nth S one-hot built on Pool instead of DVE (0=off; Pool elementwise is Q7 software — slow)


class _S:
    pass


def _prepare(row, col, vals, n_nodes, ncores, Q=NQ, gsz=GSZ):
    shard = n_nodes // ncores
    nb = (shard + 127) // 128
    ng = (nb + gsz - 1) // gsz

    qblocks = np.array_split(np.arange(nb), Q)
    q_b0 = [int(qb[0]) for qb in qblocks]
    q_b1 = [int(qb[-1]) + 1 for qb in qblocks]          # exclusive block end
    q_r0 = [b0 * 128 for b0 in q_b0]
    q_r1 = [min(b1 * 128, shard) for b1 in q_b1]
    rq = [r1 - r0 for r0, r1 in zip(q_r0, q_r1)]        # real rows per quadrant

    c = row // shard
    r = row % shard
    lb = r // 128
    dst = r % 128
    lbg = lb // gsz

    c_s = col // shard
    r_s = col % shard
    q_s = np.searchsorted(np.asarray(q_r0[1:]), r_s, side="right")
    loc = c_s * np.asarray(rq)[q_s] + (r_s - np.asarray(q_r0)[q_s])
    assert loc.max() < 32768

    order = np.lexsort((col, lb, q_s, lbg, c))
    c_o, lb_o, dst_o, q_o = c[order], lb[order], dst[order], q_s[order]
    loc_o, val_o = loc[order], vals[order]

    kk = ((c_o * ng + lb_o // gsz) * Q + q_o) * nb + lb_o
    cnt = np.zeros((ncores, nb, Q), np.int64)
    np.add.at(cnt, (c_o, lb_o, q_o), 1)
    CH = np.maximum.reduce(
        [(cnt[cc] + 127) // 128 for cc in range(ncores)])   # [nb, Q]
    for b in range(nb):
        if CH[b].sum() == 0:
            CH[b, 0] = 1

    groups = []
    tot_ch = 0
    for g0 in range(0, nb, gsz):
        blocks = list(range(g0, min(g0 + gsz, nb)))
        gi = _S()
        gi.runs = []      # per q: (ch0_abs, C, [(b, loff, chn), ...])
        for q in range(Q):
            loff = 0
            binfo = []
            ch0 = tot_ch
            for b in blocks:
                chn = int(CH[b, q])
                binfo.append((b, loff, chn))
                loff += chn
            tot_ch += loff
            gi.runs.append((ch0, loff, binfo))
        gi.blocks = blocks
        groups.append(gi)

    sched = _S()
    sched.shard, sched.nb, sched.ng, sched.Q, sched.gsz = shard, nb, ng, Q, gsz
    sched.tot_ch, sched.groups = tot_ch, groups
    sched.q_b0, sched.q_b1, sched.q_r0, sched.q_r1, sched.rq = q_b0, q_b1, q_r0, q_r1, rq
    # group index after which each quadrant's dst blocks are fully aggregated
    sched.q_done_g = [min(ng - 1, (b1 + gsz - 1) // gsz - 1) for b1 in q_b1]

    # chunk-order bounds per (core, group, q, block)
    per_core = []
    for cc in range(ncores):
        idx = np.zeros(tot_ch * 128, np.int16)
        dstv = np.zeros(tot_ch * 128, np.float32)
        valv = np.zeros(tot_ch * 128, np.float32)
        for gi in groups:
            for q in range(Q):
                ch0, C, binfo = gi.runs[q]
                for b, loff, chn in binfo:
                    if chn == 0:
                        continue
                    k2 = ((cc * ng + b // gsz) * Q + q) * nb + b
                    s, e = np.searchsorted(kk, [k2, k2 + 1])
                    n = e - s
                    o = (ch0 + loff) * 128
                    if n > 0:
                        idx[o:o+n] = loc_o[s:e].astype(np.int16)
                        dstv[o:o+n] = dst_o[s:e].astype(np.float32)
                        valv[o:o+n] = val_o[s:e].astype(np.float32)
        pc = _S()
        pc.idx_sb = np.tile(np.ascontiguousarray(idx.reshape(-1, 16).T), (8, 1))
        pc.dst_sb = np.ascontiguousarray(dstv.reshape(tot_ch, 128).T)
        pc.val_sb = np.ascontiguousarray(valv.reshape(tot_ch, 128).T)
        per_core.append(pc)
    return sched, per_core


def _build(sched, n_nodes, ncores, enable_asserts=False, passes=1):
    nb, shard, tot_ch = sched.nb, sched.shard, sched.tot_ch
    Q, rq = sched.Q, sched.rq
    npad = nb * 128

    nc = bacc.Bacc(
        "TRN2",
        target_bir_lowering=False,
        debug=False,
        enable_asserts=enable_asserts,
        num_devices=ncores,
        dynamic_dma_scratch_size=SCRATCH,
        num_swdge_queues=NQUEUES,
    )

    xtbl_d = nc.declare_dram_parameter("xtbl", [n_nodes, 128], FP16, isOutput=False)
    idx_d = nc.declare_dram_parameter("idx", [128, tot_ch * 8], I16, isOutput=False)
    dst_d = nc.declare_dram_parameter("dst", [128, tot_ch], F32, isOutput=False)
    val_d = nc.declare_dram_parameter("val", [128, tot_ch], F32, isOutput=False)
    iota_d = nc.declare_dram_parameter("iota", [128, 128], FP16, isOutput=False)
    w_d = nc.declare_dram_parameter("w", [128, 3, 128], FP16, isOutput=False)
    b_d = nc.declare_dram_parameter("b", [128, 3], F32, isOutput=False)
    fw1_d = nc.declare_dram_parameter("fw1", [128, 3, 128], FP16, isOutput=False)
    fb1_d = nc.declare_dram_parameter("fb1", [128, 1], F32, isOutput=False)
    fw2_d = nc.declare_dram_parameter("fw2", [128, 64], FP16, isOutput=False)
    fb2_d = nc.declare_dram_parameter("fb2", [64, 1], F32, isOutput=False)
    fw3_d = nc.declare_dram_parameter("fw3", [64, 2], FP16, isOutput=False)
    fb3_d = nc.declare_dram_parameter("fb3", [2, 1], F32, isOutput=False)
    out_d = nc.declare_dram_parameter("out", [2, npad], F32, isOutput=True)

    Relu = mybir.ActivationFunctionType.Relu
    Copy = mybir.ActivationFunctionType.Copy
    Ident = mybir.ActivationFunctionType.Identity
    iseq = mybir.AluOpType.is_equal
    mult = mybir.AluOpType.mult

    qstate = [0]

    with tile.TileContext(nc) as tc:
        with (
            tc.tile_pool(name="const", bufs=1) as cpool,
            tc.tile_pool(name="dram", bufs=1, space="DRAM") as dpool,
            tc.tile_pool(name="work", bufs=3) as wpool,
            tc.tile_pool(name="sbuild", bufs=SPOOL_BUFS) as spool,
            tc.tile_pool(name="psum", bufs=2, space="PSUM") as ppool,
        ):
            def load(d, shape, dtype, name):
                t = cpool.tile(list(shape), dtype, name=name)
                nc.sync.dma_start(t[:], d[:])
                return t

            idxs = load(idx_d, [128, tot_ch * 8], I16, "idxs")
            dstv = load(dst_d, [128, tot_ch], F32, "dstv")
            valv = load(val_d, [128, tot_ch], F32, "valv")
            iota = load(iota_d, [128, 128], FP16, "iota")
            w = load(w_d, [128, 3, 128], FP16, "w")
            bl = load(b_d, [128, 3], F32, "bl")
            fw1 = load(fw1_d, [128, 3, 128], FP16, "fw1")
            fb1 = load(fb1_d, [128, 1], F32, "fb1")
            fw2 = load(fw2_d, [128, 64], FP16, "fw2")
            fb2 = load(fb2_d, [64, 1], F32, "fb2")
            fw3 = load(fw3_d, [64, 2], FP16, "fw3")
            fb3 = load(fb3_d, [2, 1], F32, "fb3")

            for _pass in range(passes):
                gT = [cpool.tile([128, npad], FP16, name=f"gT{l}", tag=f"gT{l}")
                      for l in range(3)]
                outT = cpool.tile([2, npad], F32, tag="outT", name="outT")
                qoff = np.cumsum([0] + [ncores * r for r in rq]).tolist()
                ltbl = lambda l, q: (xtbl_d[qoff[q]:qoff[q] + ncores * rq[q], :]
                                     if l == 0 else tblq[l][q][:, :])
                supq = [[dpool.tile([rq[q], 128], FP16,
                                    name=f"sup{l}q{q}_{_pass}",
                                    tag=f"sup{l}q{q}_{_pass}")
                         for q in range(Q)] for l in range(1, 3)]
                tblq = [[dpool.tile([ncores * rq[q], 128], FP16,
                                    addr_space="Shared",
                                    name=f"tbl{l}q{q}_{_pass}",
                                    tag=f"tbl{l}q{q}_{_pass}")
                         for q in range(Q)] for l in range(1, 3)]
                supq = dict(zip((1, 2), supq))
                tblq = dict(zip((1, 2), tblq))

                def emit_sup(l, q):
                    """sup GEMMs for quadrant q of layer l."""
                    prev = gT[l - 1]
                    r0 = sched.q_r0[q]
                    for ib in range(sched.q_b0[q], sched.q_b1[q]):
                        ps = ppool.tile([128, 128], F32, tag="sup", name="ps_sup")
                        nc.tensor.matmul(
                            ps[:], prev[:, ib * 128:(ib + 1) * 128], w[:, l, :],
                            start=True, stop=True)
                        sup_sb = wpool.tile([128, 128], FP16, tag="sup_sb",
                                            name="sup_sb")
                        nc.scalar.activation(sup_sb[:], ps[:], Copy)
                        rows = min(128, shard - ib * 128)
                        lo = ib * 128 - r0
                        nc.sync.dma_start(
                            supq[l][q][lo:lo + rows, :], sup_sb[:rows, :])

                def emit_ag(l, q):
                    # The collective is a Pool-queue instruction: its wait for
                    # the sup DMAs stalls every dma_gather queued behind it, so
                    # its emission point is deferred (AG_DELAY groups after the
                    # sup emission; q1 between the next layer's q0/q1 runs) to
                    # when the dependency is already satisfied.
                    nc.gpsimd.collective_compute(
                        "AllGather", mybir.AluOpType.bypass,
                        replica_groups=[list(range(ncores))],
                        ins=[supq[l][q].opt()], outs=[tblq[l][q].opt()])

                def emit_fc(ib):
                    sl = slice(ib * 128, (ib + 1) * 128)
                    ps1 = ppool.tile([128, 128], F32, tag="fc1", name="ps_fc1",
                                     bufs=1)
                    for j in range(3):
                        nc.tensor.matmul(ps1[:], fw1[:, j, :], gT[j][:, sl],
                                         start=(j == 0), stop=(j == 2))
                    h1 = wpool.tile([128, 128], FP16, tag="h1", name="h1")
                    nc.scalar.activation(h1[:], ps1[:], Relu, bias=fb1[:, 0:1])
                    ps2 = ppool.tile([64, 128], F32, tag="fc2", name="ps_fc2",
                                     bufs=1)
                    nc.tensor.matmul(ps2[:], fw2[:], h1[:], start=True, stop=True)
                    h2 = wpool.tile([64, 128], FP16, tag="h2", name="h2")
                    nc.scalar.activation(h2[:], ps2[:], Relu, bias=fb2[:])
                    ps3 = ppool.tile([2, 128], F32, tag="fc3", name="ps_fc3",
                                     bufs=1)
                    nc.tensor.matmul(ps3[:], fw3[:], h2[:], start=True, stop=True)
                    nc.scalar.activation(outT[:, sl], ps3[:], Ident, bias=fb3[:])

                ng = sched.ng
                sup_at, ag_post, ag_pre = {}, {}, {}
                for l in range(2):
                    for q in range(Q):
                        gs = sched.q_done_g[q]
                        sup_at.setdefault((l, gs), []).append((l + 1, q))
                        if gs + AG_DELAY <= ng - 1:
                            ag_post.setdefault((l, gs + AG_DELAY),
                                               []).append((l + 1, q))
                        else:
                            ag_pre.setdefault((l + 1, 0, q), []).append((l + 1, q))

                for l in range(3):
                    for gidx, gi in enumerate(sched.groups):
                        mts = [None] * Q
                        for q in range(Q):
                            for (al, aq) in ag_pre.get((l, gidx, q), []):
                                emit_ag(al, aq)
                            ch0, C, binfo = gi.runs[q]
                            if C == 0:
                                continue
                            m = wpool.tile([128, C * 128], FP16, tag=f"msgs{q}",
                                           name=f"msgs{q}", bufs=MSGS_BUFS)
                            m3d = m[:].rearrange("p (c e) -> p c e", e=128)
                            for c0 in range(0, C, MAXG):
                                cn = min(MAXG, C - c0)
                                nc.gpsimd.dma_gather(
                                    out_ap=m3d[:, c0:c0 + cn, :],
                                    in_ap=ltbl(l, q),
                                    idxs_ap=idxs[:, (ch0 + c0) * 8:
                                                 (ch0 + c0 + cn) * 8],
                                    num_idxs=cn * 128, num_idxs_reg=cn * 128,
                                    elem_size=128, queue_num=qstate[0],
                                    single_packet=SP)
                                qstate[0] = (qstate[0] + 1) % NQUEUES
                            mts[q] = m
                        for bi, b in enumerate(gi.blocks):
                            ps = ppool.tile([128, 128], F32, tag="agg",
                                            name="ps_agg")
                            total = sum(gi.runs[q][2][bi][2] for q in range(Q))
                            k = 0
                            for q in range(Q):
                                ch0, C, binfo = gi.runs[q]
                                _, loff, chn = binfo[bi]
                                if chn == 0:
                                    continue
                                m3 = mts[q][:].rearrange("p (c e) -> p c e",
                                                         e=128)
                                for i in range(chn):
                                    S = spool.tile([128, 128], FP16, tag="S",
                                                   name="S")
                                    seng = (nc.gpsimd if S_POOL_EVERY and
                                            (ch0 + loff + i) % S_POOL_EVERY == 0
                                            else nc.vector)
                                    seng.tensor_scalar(
                                        S[:], iota[:],
                                        dstv[:, ch0 + loff + i:ch0 + loff + i + 1],
                                        valv[:, ch0 + loff + i:ch0 + loff + i + 1],
                                        iseq, mult)
                                    nc.tensor.matmul(
                                        ps[:], m3[:, loff + i, :], S[:],
                                        start=(k == 0), stop=(k == total - 1))
                                    k += 1
                            if l == 0:
                                # layer 1 reassociated: (A.X) @ W1
                                ax = wpool.tile([128, 128], FP16, tag="ax",
                                                name="ax")
                                nc.scalar.activation(ax[:], ps[:], Copy)
                                ps0 = ppool.tile([128, 128], F32, tag="sup",
                                                 name="ps_axw")
                                nc.tensor.matmul(ps0[:], w[:, 0, :], ax[:],
                                                 start=True, stop=True)
                                ps = ps0
                            nc.scalar.activation(
                                gT[l][:, b * 128:(b + 1) * 128], ps[:], Relu,
                                bias=bl[:, l:l + 1])
                            if l == 2:
                                emit_fc(b)
                        if l < 2:
                            for (al, aq) in sup_at.get((l, gidx), []):
                                emit_sup(al, aq)
                            for (al, aq) in ag_post.get((l, gidx), []):
                                emit_ag(al, aq)

                nc.sync.dma_start(out_d[:], outT[:])

    nc.compile()
    return nc


def _in_maps(inputs, sched, per_core, n_nodes, ncores):
    shard = sched.shard
    X = np.asarray(inputs["input_feature"], np.float32).astype(np.float16)
    # X permuted into the AllGather-output layout: q-major, core-major
    xtbl = np.concatenate(
        [X[c * shard + sched.q_r0[q]:c * shard + sched.q_r1[q]]
         for q in range(sched.Q) for c in range(ncores)], axis=0)
    xtbl = np.ascontiguousarray(xtbl)

    f16 = lambda a: np.ascontiguousarray(np.asarray(a, np.float32).astype(np.float16))
    f32 = lambda a: np.ascontiguousarray(np.asarray(a, np.float32))
    com = {
        "iota": np.ascontiguousarray(
            np.broadcast_to(np.arange(128, dtype=np.float16), (128, 128))),
        "w": np.stack([f16(inputs[k]) for k in ("W1", "W2", "W3")], axis=1),
        "b": np.stack([f32(inputs[k]) for k in ("b1", "b2", "b3")], axis=1),
        "fw1": np.ascontiguousarray(
            f16(inputs["fcW1"]).reshape(3, 128, 128).transpose(1, 0, 2)),
        "fb1": f32(inputs["fcb1"]).reshape(128, 1),
        "fw2": f16(inputs["fcW2"]),
        "fb2": f32(inputs["fcb2"]).reshape(64, 1),
        "fw3": f16(inputs["fcW3"]),
        "fb3": f32(inputs["fcb3"]).reshape(2, 1),
    }
    maps = []
    for c in range(ncores):
        m = dict(com)
        m["xtbl"] = xtbl
        m["idx"] = per_core[c].idx_sb
        m["dst"] = per_core[c].dst_sb
        m["val"] = per_core[c].val_sb
        maps.append(m)
    return maps


def _postprocess(results, sched, ncores):
    shard = sched.shard
    outs = [np.asarray(results[c]["out"], np.float32)[:, :shard].T
            for c in range(ncores)]
    return np.ascontiguousarray(np.concatenate(outs, axis=0))


_CACHE = {}


def _run(inputs, n_nodes, ncores, runner=None, enable_asserts=False, trace=False):
    row = np.asarray(inputs["adj_row"]).astype(np.int64)
    col = np.asarray(inputs["adj_col"]).astype(np.int64)
    vals = np.asarray(inputs["adj_vals"], np.float32)
    sched, per_core = _prepare(row, col, vals, n_nodes, ncores)
    nc = _build(sched, n_nodes, ncores, enable_asserts=enable_asserts)
    maps = _in_maps(inputs, sched, per_core, n_nodes, ncores)
    _CACHE["nc"], _CACHE["maps"], _CACHE["sched"] = nc, maps, sched
    if runner is None:
        res = run_bass_kernel_spmd(nc, maps, list(range(ncores)), trace=trace)
        results = res.results
        _CACHE["last_bench"] = res
    else:
        results = runner(nc, maps)
    return _postprocess(results, sched, ncores)


def kernel(**inputs):
    return _run(inputs, N_NODES, N_CORES)


# revision 16
# speedup vs baseline: 1432.9256x; 1.0080x over previous
"""GCN (3-layer graph conv + 3-layer MLP head) on 8 TRN2 NeuronCores.

Strategy (graph/1D-row parallel, per sharding hint):
  - Nodes row-sharded across 8 cores (6250 rows each). Per layer: local
    GEMM support = g_prev @ W on the node shard, AllGather the fp16
    support table, then each core aggregates its destination rows with
    dma_gather (neighbor rows) + one-hot scatter matmuls into f32 PSUM
    (S[e,dst]=val[e] built on DVE from a host-precomputed (dst,val)
    stream). Bias+ReLU+fp16 cast on ACT straight out of PSUM. Everything
    stays feature-major; the FC head runs the same way.
  - Layer 1 is reassociated (A@X)@W1 == A@(X@W1): X is a host input, so
    a pre-permuted replicated copy (xtbl, laid out exactly like the
    AllGather output) serves as the gather table — layer 1 needs no
    collective and its gathers start at t=0.
  - The remaining 2 AllGathers are chunked into Q=2 row-halves (each a
    Shared tile < 32768 rows, so int16 gather indices need no base
    split), and the next layer's local GEMM + AllGather half is emitted
    as soon as the destination blocks it needs are aggregated — the
    collective hides under the current layer's gather/aggregation.
  - The FC head is emitted per destination block inside layer 3's
    aggregation loop, so it overlaps the tail.
  - dma_gather calls round-robin 4 SWDGE queues with a 32KB/partition
    descriptor ring: the HBM-latency-bound random 256B reads get ~4x
    the concurrency of the single-queue default (the gather is the
    kernel's wall — ~460us/layer/core for 100k edges).

Numerics: fp16 storage / f32 PSUM accumulation -> ~2.6e-3 rel err vs
the f32 reference.
"""

import numpy as np

import concourse.bass as bass
import concourse.bacc as bacc
import concourse.mybir as mybir
import concourse.tile as tile
from concourse.bass_utils import run_bass_kernel_spmd

FP16 = mybir.dt.float16
F32 = mybir.dt.float32
I16 = mybir.dt.int16

N_NODES = 50000
N_CORES = 8
D = 128
NQ = 2        # AllGather row-chunks per layer (2: lowest chunk padding)
GSZ = 4       # dst blocks per gather group
MAXG = 6      # chunks per dma_gather call (ring capacity / packet limits)
SP = True     # single_packet (must be False for MAXG>8: >64 descs/engine)
SCRATCH = 32768
NQUEUES = 4
SPOOL_BUFS = 24
MSGS_BUFS = 3
AG_DELAY = 2  # groups between sup emission and its collective (stall hiding)
AGG_BUFS = 2  # PSUM agg accumulators in flight (banks: 2*sup+AGG+3*fc <= 8)
S_POOL_EVERY = 0   # every Nth S one-hot built on Pool instead of DVE (0=off; Pool elementwise is Q7 software — slow)


class _S:
    pass


def _prepare(row, col, vals, n_nodes, ncores, Q=NQ, gsz=GSZ):
    shard = n_nodes // ncores
    nb = (shard + 127) // 128
    ng = (nb + gsz - 1) // gsz

    qblocks = np.array_split(np.arange(nb), Q)
    q_b0 = [int(qb[0]) for qb in qblocks]
    q_b1 = [int(qb[-1]) + 1 for qb in qblocks]          # exclusive block end
    q_r0 = [b0 * 128 for b0 in q_b0]
    q_r1 = [min(b1 * 128, shard) for b1 in q_b1]
    rq = [r1 - r0 for r0, r1 in zip(q_r0, q_r1)]        # real rows per quadrant

    c = row // shard
    r = row % shard
    lb = r // 128
    dst = r % 128
    lbg = lb // gsz

    c_s = col // shard
    r_s = col % shard
    q_s = np.searchsorted(np.asarray(q_r0[1:]), r_s, side="right")
    loc = c_s * np.asarray(rq)[q_s] + (r_s - np.asarray(q_r0)[q_s])
    assert loc.max() < 32768

    order = np.lexsort((col, lb, q_s, lbg, c))
    c_o, lb_o, dst_o, q_o = c[order], lb[order], dst[order], q_s[order]
    loc_o, val_o = loc[order], vals[order]

    kk = ((c_o * ng + lb_o // gsz) * Q + q_o) * nb + lb_o
    cnt = np.zeros((ncores, nb, Q), np.int64)
    np.add.at(cnt, (c_o, lb_o, q_o), 1)
    CH = np.maximum.reduce(
        [(cnt[cc] + 127) // 128 for cc in range(ncores)])   # [nb, Q]
    for b in range(nb):
        if CH[b].sum() == 0:
            CH[b, 0] = 1

    groups = []
    tot_ch = 0
    for g0 in range(0, nb, gsz):
        blocks = list(range(g0, min(g0 + gsz, nb)))
        gi = _S()
        gi.runs = []      # per q: (ch0_abs, C, [(b, loff, chn), ...])
        for q in range(Q):
            loff = 0
            binfo = []
            ch0 = tot_ch
            for b in blocks:
                chn = int(CH[b, q])
                binfo.append((b, loff, chn))
                loff += chn
            tot_ch += loff
            gi.runs.append((ch0, loff, binfo))
        gi.blocks = blocks
        groups.append(gi)

    sched = _S()
    sched.shard, sched.nb, sched.ng, sched.Q, sched.gsz = shard, nb, ng, Q, gsz
    sched.tot_ch, sched.groups = tot_ch, groups
    sched.q_b0, sched.q_b1, sched.q_r0, sched.q_r1, sched.rq = q_b0, q_b1, q_r0, q_r1, rq
    # group index after which each quadrant's dst blocks are fully aggregated
    sched.q_done_g = [min(ng - 1, (b1 + gsz - 1) // gsz - 1) for b1 in q_b1]

    # chunk-order bounds per (core, group, q, block)
    per_core = []
    for cc in range(ncores):
        idx = np.zeros(tot_ch * 128, np.int16)
        dstv = np.zeros(tot_ch * 128, np.float32)
        valv = np.zeros(tot_ch * 128, np.float32)
        for gi in groups:
            for q in range(Q):
                ch0, C, binfo = gi.runs[q]
                for b, loff, chn in binfo:
                    if chn == 0:
                        continue
                    k2 = ((cc * ng + b // gsz) * Q + q) * nb + b
                    s, e = np.searchsorted(kk, [k2, k2 + 1])
                    n = e - s
                    o = (ch0 + loff) * 128
                    if n > 0:
                        idx[o:o+n] = loc_o[s:e].astype(np.int16)
                        dstv[o:o+n] = dst_o[s:e].astype(np.float32)
                        valv[o:o+n] = val_o[s:e].astype(np.float32)
        pc = _S()
        pc.idx_sb = np.tile(np.ascontiguousarray(idx.reshape(-1, 16).T), (8, 1))
        pc.dst_sb = np.ascontiguousarray(dstv.reshape(tot_ch, 128).T)
        pc.val_sb = np.ascontiguousarray(valv.reshape(tot_ch, 128).T)
        per_core.append(pc)
    return sched, per_core


def _build(sched, n_nodes, ncores, enable_asserts=False, passes=1):
    nb, shard, tot_ch = sched.nb, sched.shard, sched.tot_ch
    Q, rq = sched.Q, sched.rq
    npad = nb * 128

    nc = bacc.Bacc(
        "TRN2",
        target_bir_lowering=False,
        debug=False,
        enable_asserts=enable_asserts,
        num_devices=ncores,
        dynamic_dma_scratch_size=SCRATCH,
        num_swdge_queues=NQUEUES,
    )

    xtbl_d = nc.declare_dram_parameter("xtbl", [n_nodes, 128], FP16, isOutput=False)
    idx_d = nc.declare_dram_parameter("idx", [128, tot_ch * 8], I16, isOutput=False)
    dst_d = nc.declare_dram_parameter("dst", [128, tot_ch], F32, isOutput=False)
    val_d = nc.declare_dram_parameter("val", [128, tot_ch], F32, isOutput=False)
    iota_d = nc.declare_dram_parameter("iota", [128, 128], FP16, isOutput=False)
    w_d = nc.declare_dram_parameter("w", [128, 3, 128], FP16, isOutput=False)
    b_d = nc.declare_dram_parameter("b", [128, 3], F32, isOutput=False)
    fw1_d = nc.declare_dram_parameter("fw1", [128, 3, 128], FP16, isOutput=False)
    fb1_d = nc.declare_dram_parameter("fb1", [128, 1], F32, isOutput=False)
    fw2_d = nc.declare_dram_parameter("fw2", [128, 64], FP16, isOutput=False)
    fb2_d = nc.declare_dram_parameter("fb2", [64, 1], F32, isOutput=False)
    fw3_d = nc.declare_dram_parameter("fw3", [64, 2], FP16, isOutput=False)
    fb3_d = nc.declare_dram_parameter("fb3", [2, 1], F32, isOutput=False)
    out_d = nc.declare_dram_parameter("out", [2, npad], F32, isOutput=True)

    Relu = mybir.ActivationFunctionType.Relu
    Copy = mybir.ActivationFunctionType.Copy
    Ident = mybir.ActivationFunctionType.Identity
    iseq = mybir.AluOpType.is_equal
    mult = mybir.AluOpType.mult

    qstate = [0]

    with tile.TileContext(nc) as tc:
        with (
            tc.tile_pool(name="const", bufs=1) as cpool,
            tc.tile_pool(name="dram", bufs=1, space="DRAM") as dpool,
            tc.tile_pool(name="work", bufs=3) as wpool,
            tc.tile_pool(name="sbuild", bufs=SPOOL_BUFS) as spool,
            tc.tile_pool(name="psum", bufs=2, space="PSUM") as ppool,
        ):
            def load(d, shape, dtype, name):
                t = cpool.tile(list(shape), dtype, name=name)
                nc.sync.dma_start(t[:], d[:])
                return t

            idxs = load(idx_d, [128, tot_ch * 8], I16, "idxs")
            dstv = load(dst_d, [128, tot_ch], F32, "dstv")
            valv = load(val_d, [128, tot_ch], F32, "valv")
            iota = load(iota_d, [128, 128], FP16, "iota")
            w = load(w_d, [128, 3, 128], FP16, "w")
            bl = load(b_d, [128, 3], F32, "bl")
            fw1 = load(fw1_d, [128, 3, 128], FP16, "fw1")
            fb1 = load(fb1_d, [128, 1], F32, "fb1")
            fw2 = load(fw2_d, [128, 64], FP16, "fw2")
            fb2 = load(fb2_d, [64, 1], F32, "fb2")
            fw3 = load(fw3_d, [64, 2], FP16, "fw3")
            fb3 = load(fb3_d, [2, 1], F32, "fb3")

            for _pass in range(passes):
                gT = [cpool.tile([128, npad], FP16, name=f"gT{l}", tag=f"gT{l}")
                      for l in range(3)]
                outT = cpool.tile([2, npad], F32, tag="outT", name="outT")
                qoff = np.cumsum([0] + [ncores * r for r in rq]).tolist()
                ltbl = lambda l, q: (xtbl_d[qoff[q]:qoff[q] + ncores * rq[q], :]
                                     if l == 0 else tblq[l][q][:, :])
                supq = [[dpool.tile([rq[q], 128], FP16,
                                    name=f"sup{l}q{q}_{_pass}",
                                    tag=f"sup{l}q{q}_{_pass}")
                         for q in range(Q)] for l in range(1, 3)]
                tblq = [[dpool.tile([ncores * rq[q], 128], FP16,
                                    addr_space="Shared",
                                    name=f"tbl{l}q{q}_{_pass}",
                                    tag=f"tbl{l}q{q}_{_pass}")
                         for q in range(Q)] for l in range(1, 3)]
                supq = dict(zip((1, 2), supq))
                tblq = dict(zip((1, 2), tblq))

                def emit_sup(l, q):
                    """sup GEMMs for quadrant q of layer l."""
                    prev = gT[l - 1]
                    r0 = sched.q_r0[q]
                    for ib in range(sched.q_b0[q], sched.q_b1[q]):
                        ps = ppool.tile([128, 128], F32, tag="sup", name="ps_sup")
                        nc.tensor.matmul(
                            ps[:], prev[:, ib * 128:(ib + 1) * 128], w[:, l, :],
                            start=True, stop=True)
                        sup_sb = wpool.tile([128, 128], FP16, tag="sup_sb",
                                            name="sup_sb")
                        nc.scalar.activation(sup_sb[:], ps[:], Copy)
                        rows = min(128, shard - ib * 128)
                        lo = ib * 128 - r0
                        nc.sync.dma_start(
                            supq[l][q][lo:lo + rows, :], sup_sb[:rows, :])

                def emit_ag(l, q):
                    # The collective is a Pool-queue instruction: its wait for
                    # the sup DMAs stalls every dma_gather queued behind it, so
                    # its emission point is deferred (AG_DELAY groups after the
                    # sup emission; q1 between the next layer's q0/q1 runs) to
                    # when the dependency is already satisfied.
                    nc.gpsimd.collective_compute(
                        "AllGather", mybir.AluOpType.bypass,
                        replica_groups=[list(range(ncores))],
                        ins=[supq[l][q].opt()], outs=[tblq[l][q].opt()])

                def emit_fc(ib):
                    sl = slice(ib * 128, (ib + 1) * 128)
                    ps1 = ppool.tile([128, 128], F32, tag="fc1", name="ps_fc1",
                                     bufs=1)
                    for j in range(3):
                        nc.tensor.matmul(ps1[:], fw1[:, j, :], gT[j][:, sl],
                                         start=(j == 0), stop=(j == 2))
                    h1 = wpool.tile([128, 128], FP16, tag="h1", name="h1")
                    nc.scalar.activation(h1[:], ps1[:], Relu, bias=fb1[:, 0:1])
                    ps2 = ppool.tile([64, 128], F32, tag="fc2", name="ps_fc2",
                                     bufs=1)
                    nc.tensor.matmul(ps2[:], fw2[:], h1[:], start=True, stop=True)
                    h2 = wpool.tile([64, 128], FP16, tag="h2", name="h2")
                    nc.scalar.activation(h2[:], ps2[:], Relu, bias=fb2[:])
                    ps3 = ppool.tile([2, 128], F32, tag="fc3", name="ps_fc3",
                                     bufs=1)
                    nc.tensor.matmul(ps3[:], fw3[:], h2[:], start=True, stop=True)
                    nc.scalar.activation(outT[:, sl], ps3[:], Ident, bias=fb3[:])

                ng = sched.ng
                sup_at, ag_post, ag_pre = {}, {}, {}
                for l in range(2):
                    for q in range(Q):
                        gs = sched.q_done_g[q]
                        sup_at.setdefault((l, gs), []).append((l + 1, q))
                        if gs + AG_DELAY <= ng - 1:
                            ag_post.setdefault((l, gs + AG_DELAY),
                                               []).append((l + 1, q))
                        else:
                            ag_pre.setdefault((l + 1, 0, q), []).append((l + 1, q))

                for l in range(3):
                    for gidx, gi in enumerate(sched.groups):
                        mts = [None] * Q
                        for q in range(Q):
                            for (al, aq) in ag_pre.get((l, gidx, q), []):
                                emit_ag(al, aq)
                            ch0, C, binfo = gi.runs[q]
                            if C == 0:
                                continue
                            m = wpool.tile([128, C * 128], FP16, tag=f"msgs{q}",
                                           name=f"msgs{q}", bufs=MSGS_BUFS)
                            m3d = m[:].rearrange("p (c e) -> p c e", e=128)
                            for c0 in range(0, C, MAXG):
                                cn = min(MAXG, C - c0)
                                nc.gpsimd.dma_gather(
                                    out_ap=m3d[:, c0:c0 + cn, :],
                                    in_ap=ltbl(l, q),
                                    idxs_ap=idxs[:, (ch0 + c0) * 8:
                                                 (ch0 + c0 + cn) * 8],
                                    num_idxs=cn * 128, num_idxs_reg=cn * 128,
                                    elem_size=128, queue_num=qstate[0],
                                    single_packet=SP)
                                qstate[0] = (qstate[0] + 1) % NQUEUES
                            mts[q] = m
                        for bi, b in enumerate(gi.blocks):
                            ps = ppool.tile([128, 128], F32, tag="agg",
                                            name="ps_agg", bufs=AGG_BUFS)
                            total = sum(gi.runs[q][2][bi][2] for q in range(Q))
                            k = 0
                            for q in range(Q):
                                ch0, C, binfo = gi.runs[q]
                                _, loff, chn = binfo[bi]
                                if chn == 0:
                                    continue
                                m3 = mts[q][:].rearrange("p (c e) -> p c e",
                                                         e=128)
                                for i in range(chn):
                                    S = spool.tile([128, 128], FP16, tag="S",
                                                   name="S")
                                    seng = (nc.gpsimd if S_POOL_EVERY and
                                            (ch0 + loff + i) % S_POOL_EVERY == 0
                                            else nc.vector)
                                    seng.tensor_scalar(
                                        S[:], iota[:],
                                        dstv[:, ch0 + loff + i:ch0 + loff + i + 1],
                                        valv[:, ch0 + loff + i:ch0 + loff + i + 1],
                                        iseq, mult)
                                    nc.tensor.matmul(
                                        ps[:], m3[:, loff + i, :], S[:],
                                        start=(k == 0), stop=(k == total - 1))
                                    k += 1
                            if l == 0:
                                # layer 1 reassociated: (A.X) @ W1
                                ax = wpool.tile([128, 128], FP16, tag="ax",
                                                name="ax")
                                nc.scalar.activation(ax[:], ps[:], Copy)
                                ps0 = ppool.tile([128, 128], F32, tag="sup",
                                                 name="ps_axw")
                                nc.tensor.matmul(ps0[:], w[:, 0, :], ax[:],
                                                 start=True, stop=True)
                                ps = ps0
                            nc.scalar.activation(
                                gT[l][:, b * 128:(b + 1) * 128], ps[:], Relu,
                                bias=bl[:, l:l + 1])
                            if l == 2:
                                emit_fc(b)
                        if l < 2:
                            for (al, aq) in sup_at.get((l, gidx), []):
                                emit_sup(al, aq)
                            for (al, aq) in ag_post.get((l, gidx), []):
                                emit_ag(al, aq)

                nc.sync.dma_start(out_d[:], outT[:])

    nc.compile()
    return nc


def _in_maps(inputs, sched, per_core, n_nodes, ncores):
    shard = sched.shard
    X = np.asarray(inputs["input_feature"], np.float32).astype(np.float16)
    # X permuted into the AllGather-output layout: q-major, core-major
    xtbl = np.concatenate(
        [X[c * shard + sched.q_r0[q]:c * shard + sched.q_r1[q]]
         for q in range(sched.Q) for c in range(ncores)], axis=0)
    xtbl = np.ascontiguousarray(xtbl)

    f16 = lambda a: np.ascontiguousarray(np.asarray(a, np.float32).astype(np.float16))
    f32 = lambda a: np.ascontiguousarray(np.asarray(a, np.float32))
    com = {
        "iota": np.ascontiguousarray(
            np.broadcast_to(np.arange(128, dtype=np.float16), (128, 128))),
        "w": np.stack([f16(inputs[k]) for k in ("W1", "W2", "W3")], axis=1),
        "b": np.stack([f32(inputs[k]) for k in ("b1", "b2", "b3")], axis=1),
        "fw1": np.ascontiguousarray(
            f16(inputs["fcW1"]).reshape(3, 128, 128).transpose(1, 0, 2)),
        "fb1": f32(inputs["fcb1"]).reshape(128, 1),
        "fw2": f16(inputs["fcW2"]),
        "fb2": f32(inputs["fcb2"]).reshape(64, 1),
        "fw3": f16(inputs["fcW3"]),
        "fb3": f32(inputs["fcb3"]).reshape(2, 1),
    }
    maps = []
    for c in range(ncores):
        m = dict(com)
        m["xtbl"] = xtbl
        m["idx"] = per_core[c].idx_sb
        m["dst"] = per_core[c].dst_sb
        m["val"] = per_core[c].val_sb
        maps.append(m)
    return maps


def _postprocess(results, sched, ncores):
    shard = sched.shard
    outs = [np.asarray(results[c]["out"], np.float32)[:, :shard].T
            for c in range(ncores)]
    return np.ascontiguousarray(np.concatenate(outs, axis=0))


_CACHE = {}


def _run(inputs, n_nodes, ncores, runner=None, enable_asserts=False, trace=False):
    row = np.asarray(inputs["adj_row"]).astype(np.int64)
    col = np.asarray(inputs["adj_col"]).astype(np.int64)
    vals = np.asarray(inputs["adj_vals"], np.float32)
    sched, per_core = _prepare(row, col, vals, n_nodes, ncores)
    nc = _build(sched, n_nodes, ncores, enable_asserts=enable_asserts)
    maps = _in_maps(inputs, sched, per_core, n_nodes, ncores)
    _CACHE["nc"], _CACHE["maps"], _CACHE["sched"] = nc, maps, sched
    if runner is None:
        res = run_bass_kernel_spmd(nc, maps, list(range(ncores)), trace=trace)
        results = res.results
        _CACHE["last_bench"] = res
    else:
        results = runner(nc, maps)
    return _postprocess(results, sched, ncores)


def kernel(**inputs):
    return _run(inputs, N_NODES, N_CORES)
